# revision 1
# baseline (speedup 1.0000x reference)
"""Causal single-head attention 1D (B=4, C=512, T=4096) on 8 TRN2 NeuronCores.

Sharding: data-parallel over (batch, query-half). Each of the 8 cores handles
one batch b = core//2 and one query-half h = core%2. Host-side, each core's
copy of x[b] has every 512-wide block permuted so that the core's 256 query
columns sit FIRST within the block ([h-half, other-half]); this makes the
program identical on all cores (the only per-core data are the x permutation
and the diagonal masks) and lets the kernel capture the query columns for the
residual/Q-projection directly out of the phase-1 x stream instead of
re-loading them from HBM.

Per core:
  phase 1: stream x[b] once; build K (bf16, [c, s] layout) and V (f32r,
           [s, c] layout) resident in SBUF, and capture x_q + bp (the
           residual base, also the Q-projection input with the adjusted bias
           bq' = bq - Wq @ bp) into a resident SBUF tile.
  phase 2: per 256-query chunk: loop over the causally-needed 128-wide key
           tiles: S = K^T-layout matmul against Q (bf16), exp on ACT, causal
           mask multiply on the diagonal tiles, accumulate V^T @ E and the
           all-ones-matrix row sums (broadcast over all 128 partitions, so
           the reciprocal runs fully parallel on DVE and needs no broadcast
           matmul) in PSUM. The next chunk's Q projection and the previous
           chunk's epilogue (h PSUM->SBUF copies, output projection,
           normalize+residual, store) are dribbled into the key-tile loop so
           the PE never waits on the ACT/DVE chains.
"""

import numpy as np

import concourse.bass as bass
import concourse.bacc as bacc
import concourse.mybir as mybir
from concourse import tile
from concourse.bass_utils import run_bass_kernel_spmd
from contextlib import ExitStack

B, C, T = 4, 512, 4096
NCORE = 8
P = 128
CT = C // P            # 4 channel tiles
NCH = T // 512         # 8 query chunks of 512
SUB = 256              # per-core queries per chunk
TQ = NCH * SUB         # 2048 queries per core
NST = T // P           # 32 key tiles
SCALE = float(C) ** -0.5

f32 = mybir.dt.float32
f32r = mybir.dt.float32r
bf16 = mybir.dt.bfloat16
AF = mybir.ActivationFunctionType
ts = bass.ts


def _build_program():
    nc = bacc.Bacc("TRN2", target_bir_lowering=False, debug=False,
                   num_devices=NCORE)

    # all partition-major so every load is a single DMA
    xbd = nc.dram_tensor("xbd", [P, CT, T], f32, kind="ExternalInput")
    wqd = nc.dram_tensor("wqd", [P, CT, C], f32, kind="ExternalInput")
    wkd = nc.dram_tensor("wkd", [P, CT, C], f32, kind="ExternalInput")
    wvd = nc.dram_tensor("wvd", [P, CT, C], f32, kind="ExternalInput")
    wpd = nc.dram_tensor("wpd", [P, CT, C], f32, kind="ExternalInput")
    scd = nc.dram_tensor("scd", [P, 3 * CT], f32, kind="ExternalInput")
    bvd = nc.dram_tensor("bvd", [P, C], f32, kind="ExternalInput")
    mkd = nc.dram_tensor("mkd", [P, 4 * SUB], f32, kind="ExternalInput")
    oned = nc.dram_tensor("oned", [P, P], f32, kind="ExternalInput")
    out = nc.dram_tensor("out", [CT, P, TQ], f32, kind="ExternalOutput")

    with tile.TileContext(nc) as tc, ExitStack() as ctx:
        const = ctx.enter_context(tc.tile_pool(name="const", bufs=1))

        wq_sb = const.tile([P, CT, C], f32r, tag="wq")
        wp_sb = const.tile([P, CT, C], f32r, tag="wp")
        k_sb = const.tile([P, CT, T], bf16, tag="k")
        v_sb = const.tile([P, NST, C], f32r, tag="v")
        xq_sb = const.tile([P, CT, TQ], f32r, tag="xq")
        mask_sb = const.tile([P, 4 * SUB], f32r, tag="mask")
        bvb_sb = const.tile([P, C], f32, tag="bvb")
        sc_sb = const.tile([P, 3 * CT], f32, tag="scs")
        ones_sb = const.tile([P, P], f32r, tag="ones")

        bk_sb = sc_sb[:, 0:CT]
        bp_sb = sc_sb[:, CT:2 * CT]
        bq_sb = sc_sb[:, 2 * CT:3 * CT]

        # phase-2 constants, loaded while phase-1 compute runs; all weight
        # and constant traffic is issued from the ACT hwdge queue so the
        # sync queue carries nothing but the x stream (parallel DMA rings).
        # NOTE: do not add a third (gpsimd) ring — activating it costs
        # ~7-12us of NEFF startup, more than it saves mid-loop.
        late_dmas = [
            lambda: nc.scalar.dma_start(wq_sb[:], wqd[:].bitcast(f32r)),
            lambda: nc.scalar.dma_start(mask_sb[:], mkd[:].bitcast(f32r)),
            lambda: nc.scalar.dma_start(ones_sb[:], oned[:].bitcast(f32r)),
            lambda: nc.scalar.dma_start(wp_sb[:], wpd[:].bitcast(f32r)),
        ]

        pp = ctx.enter_context(tc.tile_pool(name="pp", bufs=3, space="PSUM"))
        ph = ctx.enter_context(tc.tile_pool(name="ph", bufs=1, space="PSUM"))

        # ---- phase 1: K, V and query-x resident in SBUF -----------------
        with tc.tile_pool(name="xp", bufs=2) as xp, \
             tc.tile_pool(name="wkv", bufs=1) as wkv:
            wk_sb = wkv.tile([P, CT, C], f32r, tag="wk")
            wv_sb = wkv.tile([P, CT, C], f32r, tag="wv")
            # the first K matmul only needs wk[:, :, 0:P] + biases + x chunk
            nc.scalar.dma_start(sc_sb[:], scd[:])
            nc.scalar.dma_start(wk_sb[:, :, 0:P], wkd[:][:, :, 0:P].bitcast(f32r))
            nc.scalar.dma_start(wk_sb[:, :, P:C],
                                wkd[:][:, :, P:C].bitcast(f32r))
            # wv/bvb stay on the ACT queue: the gpsimd DMA ring is slow to
            # warm up and these gate the first V matmuls (measured +8us)
            for cj in range(CT):
                nc.scalar.dma_start(wv_sb[:, cj, :], wvd[:][:, cj, :].bitcast(f32r))
            nc.scalar.dma_start(bvb_sb[:], bvd[:])
            for sc in range(NCH):
                xt = xp.tile([P, CT, 512], f32r, tag="xt")
                nc.sync.dma_start(xt[:],
                                  xbd[:][:, :, ts(sc, 512)].bitcast(f32r))
                for o in range(CT):
                    pk = pp.tile([P, 512], f32, tag="mm")
                    for cj in range(CT):
                        nc.tensor.matmul(pk[:], wk_sb[:, cj, ts(o, P)],
                                         xt[:, cj, :],
                                         start=(cj == 0), stop=(cj == CT - 1))
                    nc.scalar.activation(k_sb[:, o, ts(sc, 512)], pk[:],
                                         AF.Identity, bias=bk_sb[:, o:o + 1])
                for ss in range(4):
                    pv = pp.tile([P, 512], f32, tag="mm")
                    for cj in range(CT):
                        nc.tensor.matmul(pv[:], xt[:, cj, ts(ss, P)],
                                         wv_sb[:, cj, :],
                                         start=(cj == 0), stop=(cj == CT - 1))
                    nc.vector.tensor_add(v_sb[:, sc * 4 + ss, :], pv[:],
                                         bvb_sb[:])
                # capture this core's query columns (+bp) for phase 2; after
                # the K copies so they don't delay the PSUM recycling
                for j in range(CT):
                    nc.scalar.activation(xq_sb[:, j, ts(sc, SUB)],
                                         xt[:, j, 0:SUB].bitcast(f32),
                                         AF.Identity, bias=bp_sb[:, j:j + 1])
                if sc >= 1 and late_dmas:
                    late_dmas.pop(0)()
            while late_dmas:
                late_dmas.pop(0)()

        # ---- phase 2: attention per query chunk, software-pipelined -----
        with tc.tile_pool(name="qp", bufs=2) as qp, \
             tc.tile_pool(name="ep", bufs=4) as ep, \
             tc.tile_pool(name="hp", bufs=2) as hp, \
             tc.tile_pool(name="op", bufs=3) as op:

            chunk_q = {}

            def qproj_tile(c, q_sb, o):
                pq = pp.tile([P, SUB], f32, tag="mm", name="pq")
                for cj in range(CT):
                    nc.tensor.matmul(
                        pq[:], wq_sb[:, cj, ts(o, P)],
                        xq_sb[:, cj, ts(c, SUB)],
                        start=(cj == 0), stop=(cj == CT - 1))
                nc.scalar.activation(q_sb[:, o, :], pq[:], AF.Identity,
                                     bias=bq_sb[:, o:o + 1])

            def qproj(c):
                q_sb = qp.tile([P, CT, SUB], bf16, tag="q", name="q_sb")
                for o in range(CT):
                    qproj_tile(c, q_sb, o)
                chunk_q[c] = q_sb

            def s_loop(c, fin):
                """fin: list of deferred epilogue thunks for chunk c-1
                (h-copy burst first, then per-o projection+store), dribbled
                into this chunk's key-tile loop."""
                q_sb = chunk_q.pop(c)
                ntr = 4 * c + 4
                # h PSUM->SBUF copies of the previous chunk go first (DVE is
                # idle here and the first EV matmul write-waits these banks)
                if fin:
                    fin.pop(0)()
                # separate PSUM tiles per accumulation group: the PSUM
                # pending-zero state from a matmul's start flag is tracked
                # per tensor/bank, so interleaved groups can't share a tile
                ht = [ph.tile([P, SUB], f32, tag=f"ht{cs}", name=f"ht{cs}")
                      for cs in range(CT)]
                sm = ph.tile([P, SUB], f32, tag="sm", name="sm")
                st_tiles = {}

                def qk(k):
                    stp = pp.tile([P, SUB], f32, tag="mm", name="stp")
                    for cj in range(CT):
                        nc.tensor.matmul(stp[:], k_sb[:, cj, ts(k, P)],
                                         q_sb[:, cj, :],
                                         start=(cj == 0), stop=(cj == CT - 1))
                    st_tiles[k] = stp

                # the NEXT chunk's Q projection is dribbled one output tile
                # per key-tile iteration: its PSUM slots then recycle at the
                # exp pace and its ACT copies stay off the chunk epilogue
                nq = qp.tile([P, CT, SUB], bf16, tag="q", name="q_sb") \
                    if c + 1 < NCH else None
                qoff = 1 if ntr > 4 else 0

                ets = {}

                def sum_mm(k):
                    # sm matmuls run 2 tiles late so the first one is not
                    # write-blocked on the previous chunk's reciprocal
                    nc.tensor.matmul(sm[:], ones_sb[:], ets.pop(k)[:],
                                     start=(k == 0), stop=(k == ntr - 1))

                qk(0)
                for k in range(ntr):
                    if k + 1 < ntr:
                        qk(k + 1)
                    if nq is not None and qoff <= k < qoff + CT:
                        qproj_tile(c + 1, nq, k - qoff)
                    elif fin and k >= qoff + CT:
                        fin.pop(0)()
                    stp = st_tiles.pop(k)
                    et = ep.tile([P, SUB], f32r, tag="et", name="et")
                    nc.scalar.activation(et[:], stp[:], AF.Exp, scale=SCALE)
                    if k >= 4 * c:
                        nc.vector.tensor_mul(et[:], et[:],
                                             mask_sb[:, ts(k - 4 * c, SUB)])
                    for cs in range(CT):
                        nc.tensor.matmul(ht[cs][:], v_sb[:, k, ts(cs, P)],
                                         et[:], start=(k == 0),
                                         stop=(k == ntr - 1))
                    ets[k] = et
                    if k >= 2:
                        sum_mm(k - 2)
                for k in range(max(0, ntr - 2), ntr):
                    sum_mm(k)
                if nq is not None:
                    chunk_q[c + 1] = nq
                while fin:
                    fin.pop(0)()
                # row sums were accumulated broadcast across all partitions,
                # so the reciprocal runs 128-way parallel and the result
                # multiplies the output projection directly
                r_sb = op.tile([P, SUB], f32r, tag="rsb", name="r_sb")
                with nc.allow_low_precision(reason="float32r is fp32-width"):
                    nc.vector.reciprocal(r_sb[:], sm[:])
                return ht, r_sb

            def finish_thunks(c, ht, r_sb, last=False):
                hs = hp.tile([P, CT, SUB], f32r, tag="hs", name="hs")

                def copy_h():
                    # mid-pipeline ACT is busy with the next chunk's exps,
                    # so the copies run on DVE; for the final chunk both
                    # engines are idle and splitting halves the latency
                    for cs in range(CT):
                        if last and cs >= 2:
                            nc.scalar.activation(hs[:, cs, :], ht[cs][:],
                                                 AF.Identity)
                        else:
                            nc.vector.tensor_copy(hs[:, cs, :], ht[cs][:])

                def proj_o(o):
                    pu = pp.tile([P, SUB], f32, tag="mm", name="pu")
                    for cj in range(CT):
                        nc.tensor.matmul(pu[:], wp_sb[:, cj, ts(o, P)],
                                         hs[:, cj, :],
                                         start=(cj == 0), stop=(cj == CT - 1))
                    og = op.tile([P, SUB], f32, tag="og", name="og")
                    nc.vector.tensor_mul(og[:], pu[:], r_sb[:])
                    nc.vector.tensor_add(og[:], og[:],
                                         xq_sb[:, o, ts(c, SUB)].bitcast(f32))
                    nc.sync.dma_start(out[o][:, ts(c, SUB)], og[:])

                return [copy_h] + [lambda o=o: proj_o(o) for o in range(CT)]

            qproj(0)
            fin = []
            for c in range(NCH):
                ht, r_sb = s_loop(c, fin)
                fin = finish_thunks(c, ht, r_sb, last=(c == NCH - 1))
            while fin:
                fin.pop(0)()

    nc.finalize()
    return nc


def _masks(h):
    m = np.zeros((4, P, SUB), np.float32)
    f = np.arange(SUB)[None, :]
    p = np.arange(P)[:, None]
    m[0] = (f >= p).astype(np.float32)
    m[1] = (f >= p + 128).astype(np.float32)
    if h == 1:
        m[2] = 1.0
        m[3] = 1.0
    return m


def _pmajor(w):
    # [C_out, C_in] weight (transposed use) -> [P, CT, C] partition-major
    return np.ascontiguousarray(
        w.T.reshape(CT, P, C).transpose(1, 0, 2))


def _in_maps(inputs):
    x = np.asarray(inputs["x"], np.float32)
    Wq = np.asarray(inputs["Wq"], np.float32)
    bq = np.asarray(inputs["bq"], np.float32)
    Wk = np.asarray(inputs["Wk"], np.float32)
    bk = np.asarray(inputs["bk"], np.float32)
    Wv = np.asarray(inputs["Wv"], np.float32)
    bv = np.asarray(inputs["bv"], np.float32)
    Wp = np.asarray(inputs["Wp"], np.float32)
    bp = np.asarray(inputs["bp"], np.float32)

    bq_adj = bq - Wq @ bp  # Q is projected from (x + bp)
    scd = np.concatenate([bk.reshape(CT, P).T, bp.reshape(CT, P).T,
                          bq_adj.reshape(CT, P).T], axis=1)
    common = {
        "wqd": _pmajor(Wq),
        "wkd": _pmajor(Wk),
        "wvd": _pmajor(Wv),
        "wpd": _pmajor(Wp),
        "scd": np.ascontiguousarray(scd),
        "bvd": np.ascontiguousarray(np.broadcast_to(bv[None, :], (P, C))),
        "oned": np.ones((P, P), np.float32),
    }
    maps = []
    for core in range(NCORE):
        b, h = divmod(core, 2)
        # per-512-block permutation: this core's query half first
        perm = (np.arange(NCH)[:, None] * 512
                + (h * SUB + np.arange(512)[None, :]) % 512).ravel()
        cols = (np.arange(NCH)[:, None] * 512 + h * SUB
                + np.arange(SUB)[None, :]).ravel()
        m = dict(common)
        m["xbd"] = np.ascontiguousarray(
            x[b][:, perm].reshape(CT, P, T).transpose(1, 0, 2))
        m["mkd"] = np.ascontiguousarray(
            _masks(h).transpose(1, 0, 2).reshape(P, 4 * SUB))
        maps.append((m, b, cols))
    return maps


_prog_cache = {}


def _get_program():
    if "nc" not in _prog_cache:
        _prog_cache["nc"] = _build_program()
    return _prog_cache["nc"]


def kernel(**inputs):
    return _run(inputs, trace=False)[0]


def _run(inputs, trace=False):
    nc = _get_program()
    maps = _in_maps(inputs)
    res = run_bass_kernel_spmd(nc, [m for m, _, _ in maps],
                               core_ids=list(range(NCORE)), trace=trace)
    x = np.asarray(inputs["x"], np.float32)
    full = np.empty((B, C, T), np.float32)
    for core, (_, b, cols) in enumerate(maps):
        full[b][:, cols] = res.results[core]["out"].reshape(C, TQ)
    return full, res



# revision 2
# speedup vs baseline: 1.3563x; 1.3563x over previous
"""Causal single-head attention 1D (B=4, C=512, T=4096) on 8 TRN2 NeuronCores.

Sharding: data-parallel over (batch, query-half). Each of the 8 cores handles
one batch b = core//2 and one query-half h = core%2. Host-side, each core's
copy of x[b] has every 512-wide block permuted so that the core's 256 query
columns sit FIRST within the block; the program is identical on all cores.

Algebraic folding (all host-side, exact):
  S[s,t] = (Wk x_s + bk).(Wq x_t + bq) = x_s.(W~ x_t + b~) + f(t), where
  W~ = Wk^T Wq, b~ = Wk^T bq, and f(t) is constant over keys s, so it cancels
  in the causal softmax.  Hence K == raw x (no K-projection) and a single
  Q~-projection with host-precomputed W~ (the 1/sqrt(C) scale folded in).
  Likewise h = E^T V with V = Wv x + bv gives
  Wp h = (Wp Wv)(x E) + (Wp bv) * sum(E), and sum(E) * (1/sum(E)) = 1, so
  raw x^T replaces V (no V-projection), the out-projection uses W2 = Wp Wv,
  and bias2 = bp + Wp bv is a constant added on the HOST after gather (the
  residual x is also added on the host, in full f32 precision).

Per core the device program is a single software-pipelined chunk loop:
  x streams in bf16 in two layouts ([chan, t] for K/Q~-moving, [t, chan] for
  the U matmul) straight into resident SBUF; per 256-query chunk: Q~ = W~ x,
  S = K-tiles^T Q~, E = exp(S) (bf16, causal-masked on the diagonal tiles),
  U += xT-tiles^T E and the denominator row-sums accumulate via an all-ones
  matmul broadcast over all 128 partitions (so the reciprocal runs fully
  parallel on DVE).  The next chunk's Q~-projection and the previous chunk's
  epilogue (U PSUM->SBUF copies, W2-projection, normalize, store) are
  dribbled into the key-tile loop so the PE never waits on the ACT/DVE
  chains.  Output stores ride the scalar DMA ring (free after the weights),
  the x streams ride the sync ring, all prefetched from the prologue.
"""

import numpy as np
import ml_dtypes

import concourse.bass as bass
import concourse.bacc as bacc
import concourse.mybir as mybir
from concourse import tile
from concourse.bass_utils import run_bass_kernel_spmd
from contextlib import ExitStack

B, C, T = 4, 512, 4096
NCORE = 8
P = 128
CT = C // P            # 4 channel tiles
NCH = T // 512         # 8 query chunks of 512
SUB = 256              # per-core queries per chunk
TQ = NCH * SUB         # 2048 queries per core
NST = T // P           # 32 key tiles
SCALE = float(C) ** -0.5
KCH = CT * 512         # bf16 elements per partition per x chunk (both layouts)

f32 = mybir.dt.float32
f32r = mybir.dt.float32r
bf16 = mybir.dt.bfloat16
AF = mybir.ActivationFunctionType
ts = bass.ts


def _build_program():
    nc = bacc.Bacc("TRN2", target_bir_lowering=False, debug=False,
                   num_devices=NCORE)

    # chunk-outer DRAM layouts so every chunk DMA is contiguous per partition
    xkd = nc.dram_tensor("xkd", [NCH, P, KCH], bf16, kind="ExternalInput")
    xtd = nc.dram_tensor("xtd", [NCH, P, KCH], bf16, kind="ExternalInput")
    wqd = nc.dram_tensor("wqd", [P, CT, C], bf16, kind="ExternalInput")
    wpd = nc.dram_tensor("wpd", [P, CT, C], f32, kind="ExternalInput")
    scd = nc.dram_tensor("scd", [P, CT], f32, kind="ExternalInput")
    mkd = nc.dram_tensor("mkd", [P, 4 * SUB], bf16, kind="ExternalInput")
    oned = nc.dram_tensor("oned", [P, P], bf16, kind="ExternalInput")
    out = nc.dram_tensor("out", [CT, P, TQ], f32, kind="ExternalOutput")

    with tile.TileContext(nc) as tc, ExitStack() as ctx:
        const = ctx.enter_context(tc.tile_pool(name="const", bufs=1))

        k_sb = const.tile([P, NCH * KCH], bf16, tag="k")       # x, [chan, t]
        xT_sb = const.tile([P, NCH * KCH], bf16, tag="xt")     # x, [t, chan]
        wq_sb = const.tile([P, CT, C], bf16, tag="wq")
        wp_sb = const.tile([P, CT, C], f32r, tag="wp")
        mask_sb = const.tile([P, 4 * SUB], bf16, tag="mask")
        sc_sb = const.tile([P, CT], f32, tag="scs")
        ones_sb = const.tile([P, P], bf16, tag="ones")

        bq_sb = sc_sb  # b~ (adjusted Q bias) only

        # prologue DMAs: sync ring carries both x streams (front-loaded, in
        # the order phase-2 consumes them); scalar ring carries weights and
        # then the output stores.
        nc.scalar.dma_start(sc_sb[:], scd[:])
        nc.scalar.dma_start(mask_sb[:], mkd[:])
        nc.scalar.dma_start(ones_sb[:], oned[:])
        nc.scalar.dma_start(wq_sb[:], wqd[:])
        nc.sync.dma_start(k_sb[:, 0:KCH], xkd[0])
        # first xT chunk split per key tile so U(chunk 0) pipelines with it
        for j in range(4):
            nc.scalar.dma_start(xT_sb[:, ts(j, 512)], xtd[0][:, ts(j, 512)])
        nc.scalar.dma_start(wp_sb[:], wpd[:].bitcast(f32r))
        nc.sync.dma_start(k_sb[:, KCH:2 * KCH], xkd[1])
        for sc in range(1, NCH):
            nc.sync.dma_start(xT_sb[:, ts(sc, KCH)], xtd[sc])
            if sc + 1 < NCH:
                nc.sync.dma_start(k_sb[:, ts(sc + 1, KCH)], xkd[sc + 1])

        pp = ctx.enter_context(tc.tile_pool(name="pp", bufs=3, space="PSUM"))
        ph = ctx.enter_context(tc.tile_pool(name="ph", bufs=1, space="PSUM"))

        with tc.tile_pool(name="qp", bufs=2) as qp, \
             tc.tile_pool(name="ep", bufs=4) as ep, \
             tc.tile_pool(name="hp", bufs=2) as hp, \
             tc.tile_pool(name="op", bufs=3) as op:

            chunk_q = {}

            def qproj_tile(c, q_sb, o):
                pq = pp.tile([P, SUB], f32, tag="mm", name="pq")
                for cj in range(CT):
                    mv = k_sb[:, c * KCH + cj * 512: c * KCH + cj * 512 + SUB]
                    nc.tensor.matmul(
                        pq[:], wq_sb[:, cj, ts(o, P)], mv,
                        start=(cj == 0), stop=(cj == CT - 1))
                nc.scalar.activation(q_sb[:, o, :], pq[:], AF.Identity,
                                     bias=bq_sb[:, o:o + 1])

            def qproj(c):
                q_sb = qp.tile([P, CT, SUB], bf16, tag="q", name="q_sb")
                for o in range(CT):
                    qproj_tile(c, q_sb, o)
                chunk_q[c] = q_sb

            def s_loop(c, fin):
                """fin: list of deferred epilogue thunks for chunk c-1
                (h-copy burst first, then per-o projection+store), dribbled
                into this chunk's key-tile loop."""
                q_sb = chunk_q.pop(c)
                ntr = 4 * c + 4
                # U PSUM->SBUF copies of the previous chunk go first (DVE is
                # idle here and the first U matmul write-waits these banks)
                if fin:
                    fin.pop(0)()
                # separate PSUM tiles per accumulation group: the PSUM
                # pending-zero state from a matmul's start flag is tracked
                # per tensor/bank, so interleaved groups can't share a tile
                ht = [ph.tile([P, SUB], f32, tag=f"ht{cs}", name=f"ht{cs}")
                      for cs in range(CT)]
                sm = ph.tile([P, SUB], f32, tag="sm", name="sm")
                st_tiles = {}

                def qk(kk):
                    stp = pp.tile([P, SUB], f32, tag="mm", name="stp")
                    ko, kj = divmod(kk, 4)
                    base = ko * KCH + kj * P
                    for cj in range(CT):
                        nc.tensor.matmul(
                            stp[:], k_sb[:, base + cj * 512: base + cj * 512 + P],
                            q_sb[:, cj, :],
                            start=(cj == 0), stop=(cj == CT - 1))
                    st_tiles[kk] = stp

                # the NEXT chunk's Q projection is dribbled one output tile
                # per key-tile iteration: its PSUM slots then recycle at the
                # exp pace and its ACT copies stay off the chunk epilogue
                nq = qp.tile([P, CT, SUB], bf16, tag="q", name="q_sb") \
                    if c + 1 < NCH else None
                qoff = 1 if ntr > 4 else 0

                ets = {}

                def sum_mm(kk):
                    # sm matmuls run 2 tiles late so the first one is not
                    # write-blocked on the previous chunk's reciprocal
                    nc.tensor.matmul(sm[:], ones_sb[:], ets.pop(kk)[:],
                                     start=(kk == 0), stop=(kk == ntr - 1))

                qk(0)
                for k in range(ntr):
                    if k + 1 < ntr:
                        qk(k + 1)
                    if nq is not None and qoff <= k < qoff + CT:
                        qproj_tile(c + 1, nq, k - qoff)
                    elif fin and k >= qoff + CT:
                        fin.pop(0)()
                    stp = st_tiles.pop(k)
                    et = ep.tile([P, SUB], bf16, tag="et", name="et")
                    nc.scalar.activation(et[:], stp[:], AF.Exp)
                    if k >= 4 * c:
                        nc.vector.tensor_mul(et[:], et[:],
                                             mask_sb[:, ts(k - 4 * c, SUB)])
                    ko, kj = divmod(k, 4)
                    ubase = ko * KCH + kj * 512
                    for cs in range(CT):
                        nc.tensor.matmul(
                            ht[cs][:],
                            xT_sb[:, ubase + cs * P: ubase + cs * P + P],
                            et[:], start=(k == 0), stop=(k == ntr - 1))
                    ets[k] = et
                    if k >= 2:
                        sum_mm(k - 2)
                for k in range(max(0, ntr - 2), ntr):
                    sum_mm(k)
                if nq is not None:
                    chunk_q[c + 1] = nq
                while fin:
                    fin.pop(0)()
                # row sums were accumulated broadcast across all partitions,
                # so the reciprocal runs 128-way parallel and the result
                # multiplies the output projection directly
                r_sb = op.tile([P, SUB], f32r, tag="rsb", name="r_sb")
                with nc.allow_low_precision(reason="float32r is fp32-width"):
                    nc.vector.reciprocal(r_sb[:], sm[:])
                return ht, r_sb

            def finish_thunks(c, ht, r_sb, last=False):
                hs = hp.tile([P, CT, SUB], f32r, tag="hs", name="hs")

                def copy_h():
                    # mid-pipeline ACT is busy with the next chunk's exps,
                    # so the copies run on DVE; for the final chunk both
                    # engines are idle and splitting halves the latency
                    for cs in range(CT):
                        if last and cs >= 2:
                            nc.scalar.activation(hs[:, cs, :], ht[cs][:],
                                                 AF.Identity)
                        else:
                            nc.vector.tensor_copy(hs[:, cs, :], ht[cs][:])

                def proj_o(o):
                    pu = pp.tile([P, SUB], f32, tag="mm", name="pu")
                    for cj in range(CT):
                        nc.tensor.matmul(pu[:], wp_sb[:, cj, ts(o, P)],
                                         hs[:, cj, :],
                                         start=(cj == 0), stop=(cj == CT - 1))
                    og = op.tile([P, SUB], f32, tag="og", name="og")
                    nc.vector.tensor_mul(og[:], pu[:], r_sb[:])
                    nc.scalar.dma_start(out[o][:, ts(c, SUB)], og[:])

                return [copy_h] + [lambda o=o: proj_o(o) for o in range(CT)]

            qproj(0)
            fin = []
            for c in range(NCH):
                ht, r_sb = s_loop(c, fin)
                fin = finish_thunks(c, ht, r_sb, last=(c == NCH - 1))
            while fin:
                fin.pop(0)()

    nc.finalize()
    return nc


def _masks(h):
    m = np.zeros((4, P, SUB), np.float32)
    f = np.arange(SUB)[None, :]
    p = np.arange(P)[:, None]
    m[0] = (f >= p).astype(np.float32)
    m[1] = (f >= p + 128).astype(np.float32)
    if h == 1:
        m[2] = 1.0
        m[3] = 1.0
    return m


def _pmajor(w):
    # [C_out, C_in] weight (transposed use) -> [P, CT, C] partition-major
    return np.ascontiguousarray(
        w.T.reshape(CT, P, C).transpose(1, 0, 2))


def _in_maps(inputs):
    x = np.asarray(inputs["x"], np.float32)
    Wq = np.asarray(inputs["Wq"], np.float64)
    bq = np.asarray(inputs["bq"], np.float64)
    Wk = np.asarray(inputs["Wk"], np.float64)
    Wv = np.asarray(inputs["Wv"], np.float64)
    bv = np.asarray(inputs["bv"], np.float64)
    Wp = np.asarray(inputs["Wp"], np.float64)
    bp = np.asarray(inputs["bp"], np.float64)

    Wt = (Wk.T @ Wq) * SCALE           # folded Q~ weights (scale included)
    bt = (Wk.T @ bq) * SCALE           # folded Q~ bias
    W2 = Wp @ Wv                       # folded output projection
    b2 = (bp + Wp @ bv).astype(np.float32)   # host-side constant bias

    common = {
        "wqd": _pmajor(Wt.astype(np.float32)).astype(ml_dtypes.bfloat16),
        "wpd": _pmajor(W2.astype(np.float32)),
        "scd": np.ascontiguousarray(
            bt.astype(np.float32).reshape(CT, P).T),
        "oned": np.ones((P, P), ml_dtypes.bfloat16),
    }
    maps = []
    for core in range(NCORE):
        b, h = divmod(core, 2)
        # per-512-block permutation: this core's query half first
        perm = (np.arange(NCH)[:, None] * 512
                + (h * SUB + np.arange(512)[None, :]) % 512).ravel()
        cols = (np.arange(NCH)[:, None] * 512 + h * SUB
                + np.arange(SUB)[None, :]).ravel()
        xp = x[b][:, perm].astype(ml_dtypes.bfloat16)     # [C, T]
        m = dict(common)
        # [chan, t] layout, chunk-outer: xkd[sc][p, cj*512 + t'] =
        #   xp[cj*128+p, sc*512+t']
        m["xkd"] = np.ascontiguousarray(
            xp.reshape(CT, P, NCH, 512).transpose(2, 1, 0, 3)
            .reshape(NCH, P, KCH))
        # [t, chan] layout, chunk-outer: xtd[sc][p, j*C + c] =
        #   xp[c, sc*512 + j*128 + p]
        m["xtd"] = np.ascontiguousarray(
            xp.T.reshape(NCH, 4, P, C).transpose(0, 2, 1, 3)
            .reshape(NCH, P, KCH))
        m["mkd"] = np.ascontiguousarray(
            _masks(h).transpose(1, 0, 2).reshape(P, 4 * SUB)
            .astype(ml_dtypes.bfloat16))
        maps.append((m, b, cols))
    return maps, b2


_prog_cache = {}


def _get_program():
    if "nc" not in _prog_cache:
        _prog_cache["nc"] = _build_program()
    return _prog_cache["nc"]


def kernel(**inputs):
    return _run(inputs, trace=False)[0]


def _run(inputs, trace=False):
    nc = _get_program()
    maps, b2 = _in_maps(inputs)
    res = run_bass_kernel_spmd(nc, [m for m, _, _ in maps],
                               core_ids=list(range(NCORE)), trace=trace)
    x = np.asarray(inputs["x"], np.float32)
    full = np.empty((B, C, T), np.float32)
    for core, (_, b, cols) in enumerate(maps):
        full[b][:, cols] = res.results[core]["out"].reshape(C, TQ)
    # residual + folded constant bias, both in full f32 on the host
    full += x + b2[None, :, None]
    return full, res


# revision 16
# speedup vs baseline: 1.5001x; 1.1060x over previous
"""Causal single-head attention 1D (B=4, C=512, T=4096) on 8 TRN2 NeuronCores.

Sharding: data-parallel over (batch, query-half). Each of the 8 cores handles
one batch b = core//2 and one query-half h = core%2. Host-side, each core's
copy of x[b] has every 512-wide block permuted so that the core's 256 query
columns sit FIRST within the block; the program is identical on all cores.

Algebraic folding (all host-side, exact):
  S[s,t] = (Wk x_s + bk).(Wq x_t + bq) = x_s.(W~ x_t + b~) + f(t), where
  W~ = Wk^T Wq, b~ = Wk^T bq, and f(t) is constant over keys s, so it cancels
  in the causal softmax.  Hence K == raw x (no K-projection) and a single
  Q~-projection with host-precomputed W~ (the 1/sqrt(C) scale folded in).
  Likewise h = E^T V with V = Wv x + bv gives
  Wp h = (Wp Wv)(x E) + (Wp bv) * sum(E), and sum(E) * (1/sum(E)) = 1, so
  raw x^T replaces V (no V-projection), the out-projection uses W2 = Wp Wv,
  and bias2 = bp + Wp bv is a constant added on the HOST after gather (the
  residual x is also added on the host, in full f32 precision).

Per core the device program is a single software-pipelined chunk loop:
  x streams in bf16 in two layouts ([chan, t] for K/Q~-moving, [t, chan] for
  the U matmul) straight into resident SBUF; per 256-query chunk: Q~ = W~ x,
  S = K-tiles^T Q~, E = exp(S) (bf16, causal-masked on the diagonal tiles),
  U += xT-tiles^T E and the denominator row-sums accumulate via an all-ones
  matmul broadcast over all 128 partitions (so the reciprocal runs fully
  parallel on DVE).  The next chunk's Q~-projection and the previous chunk's
  epilogue (U PSUM->SBUF copies, W2-projection, normalize, store) are
  dribbled into the key-tile loop so the PE never waits on the ACT/DVE
  chains.  Output stores ride the scalar DMA ring (free after the weights),
  the x streams ride the sync ring, all prefetched from the prologue.
"""

import numpy as np
import ml_dtypes

import concourse.bass as bass
import concourse.bacc as bacc
import concourse.mybir as mybir
from concourse import tile
from concourse.bass_utils import run_bass_kernel_spmd
from contextlib import ExitStack

B, C, T = 4, 512, 4096
NCORE = 8
P = 128
CT = C // P            # 4 channel tiles
NCH = T // 512         # 8 query chunks of 512
SUB = 256              # per-core queries per chunk
TQ = NCH * SUB         # 2048 queries per core
NST = T // P           # 32 key tiles
SCALE = float(C) ** -0.5
KCH = CT * 512         # bf16 elements per partition per x chunk (both layouts)

f32 = mybir.dt.float32
f32r = mybir.dt.float32r
bf16 = mybir.dt.bfloat16
AF = mybir.ActivationFunctionType
ts = bass.ts


def _build_program():
    nc = bacc.Bacc("TRN2", target_bir_lowering=False, debug=False,
                   num_devices=NCORE)

    # chunk-outer DRAM layouts so every chunk DMA is contiguous per partition
    xkd = nc.dram_tensor("xkd", [NCH, P, KCH], bf16, kind="ExternalInput")
    xtd = nc.dram_tensor("xtd", [NCH, P, KCH], bf16, kind="ExternalInput")
    wqd = nc.dram_tensor("wqd", [P, CT, C], bf16, kind="ExternalInput")
    wpd = nc.dram_tensor("wpd", [P, CT, C], bf16, kind="ExternalInput")
    scd = nc.dram_tensor("scd", [P, CT], f32, kind="ExternalInput")
    mkd = nc.dram_tensor("mkd", [P, 4 * SUB], bf16, kind="ExternalInput")
    oned = nc.dram_tensor("oned", [P, P], f32, kind="ExternalInput")
    out = nc.dram_tensor("out", [CT, P, TQ], f32, kind="ExternalOutput")

    with tile.TileContext(nc) as tc, ExitStack() as ctx:
        const = ctx.enter_context(tc.tile_pool(name="const", bufs=1))

        k_sb = const.tile([P, NCH * KCH], bf16, tag="k")       # x, [chan, t]
        xT_sb = const.tile([P, NCH * KCH], bf16, tag="xt")     # x, [t, chan]
        wq_sb = const.tile([P, CT, C], bf16, tag="wq")
        wp_sb = const.tile([P, CT, C], bf16, tag="wp")
        mask_sb = const.tile([P, 4 * SUB], bf16, tag="mask")
        sc_sb = const.tile([P, CT], f32, tag="scs")
        ones_sb = const.tile([P, P], f32r, tag="ones")

        bq_sb = sc_sb  # b~ (adjusted Q bias) only

        # prologue DMAs, ordered by first use: W~ is split across both rings
        # so the chunk-0 Q~ projection (the critical-path head) starts as
        # early as possible; the sync ring then carries W2 and both x
        # streams interleaved in consumption order; the scalar ring carries
        # the small constants, the first xT chunk (split per key tile so
        # U(chunk 0) pipelines with it), and later the output stores.
        nc.sync.dma_start(wq_sb[:, 0:2, :], wqd[:][:, 0:2, :])
        nc.scalar.dma_start(wq_sb[:, 2:4, :], wqd[:][:, 2:4, :])
        nc.scalar.dma_start(sc_sb[:], scd[:])
        nc.scalar.dma_start(mask_sb[:], mkd[:])
        nc.scalar.dma_start(ones_sb[:], oned[:].bitcast(f32r))
        nc.sync.dma_start(k_sb[:, 0:KCH], xkd[0])
        for j in range(4):
            nc.scalar.dma_start(xT_sb[:, ts(j, 512)], xtd[0][:, ts(j, 512)])
        nc.sync.dma_start(k_sb[:, KCH:2 * KCH], xkd[1])
        nc.sync.dma_start(wp_sb[:], wpd[:])
        for sc in range(1, NCH):
            nc.sync.dma_start(xT_sb[:, ts(sc, KCH)], xtd[sc])
            if sc + 1 < NCH:
                nc.sync.dma_start(k_sb[:, ts(sc + 1, KCH)], xkd[sc + 1])

        pp = ctx.enter_context(tc.tile_pool(name="pp", bufs=3, space="PSUM"))
        ph = ctx.enter_context(tc.tile_pool(name="ph", bufs=1, space="PSUM"))

        with tc.tile_pool(name="qp", bufs=2) as qp, \
             tc.tile_pool(name="ep", bufs=4) as ep, \
             tc.tile_pool(name="ap", bufs=2) as ap, \
             tc.tile_pool(name="hp", bufs=2) as hp, \
             tc.tile_pool(name="op", bufs=3) as op:

            chunk_q = {}

            def qproj_tile(c, q_sb, o):
                pq = pp.tile([P, SUB], f32, tag="mm", name="pq")
                for cj in range(CT):
                    mv = k_sb[:, c * KCH + cj * 512: c * KCH + cj * 512 + SUB]
                    nc.tensor.matmul(
                        pq[:], wq_sb[:, cj, ts(o, P)], mv,
                        start=(cj == 0), stop=(cj == CT - 1))
                nc.scalar.activation(q_sb[:, o, :], pq[:], AF.Identity,
                                     bias=bq_sb[:, o:o + 1])

            def qproj(c):
                q_sb = qp.tile([P, CT, SUB], bf16, tag="q", name="q_sb")
                for o in range(CT):
                    qproj_tile(c, q_sb, o)
                chunk_q[c] = q_sb

            def s_loop(c, fin):
                """fin: list of deferred epilogue thunks for chunk c-1
                (h-copy burst first, then per-o projection+store), dribbled
                into this chunk's key-tile loop."""
                q_sb = chunk_q.pop(c)
                ntr = 4 * c + 4
                # U PSUM->SBUF copies of the previous chunk go first (DVE is
                # idle here)
                if fin:
                    fin.pop(0)()
                # separate PSUM tiles per accumulation group: the PSUM
                # pending-zero state from a matmul's start flag is tracked
                # per tensor/bank, so interleaved groups can't share a tile
                ht = [ph.tile([P, SUB], f32, tag=f"ht{cs}", name=f"ht{cs}")
                      for cs in range(CT)]
                sm = ph.tile([P, SUB], f32, tag="sm", name="sm")
                # denominator: E tiles accumulate on DVE (off the PE), one
                # broadcast ones-matmul per chunk turns the per-key partial
                # sums into the 128-partition-replicated row sum
                acc = ap.tile([P, SUB], f32r, tag="acc", name="acc")
                st_tiles = {}

                def qk(kk):
                    stp = pp.tile([P, SUB], f32, tag="mm", name="stp")
                    ko, kj = divmod(kk, 4)
                    base = ko * KCH + kj * P
                    for cj in range(CT):
                        nc.tensor.matmul(
                            stp[:], k_sb[:, base + cj * 512: base + cj * 512 + P],
                            q_sb[:, cj, :],
                            start=(cj == 0), stop=(cj == CT - 1))
                    st_tiles[kk] = stp

                # the NEXT chunk's Q projection is dribbled one output tile
                # per key-tile iteration: its PSUM slots then recycle at the
                # exp pace and its ACT copies stay off the chunk epilogue
                nq = qp.tile([P, CT, SUB], bf16, tag="q", name="q_sb") \
                    if c + 1 < NCH else None
                qoff = 1 if ntr > 4 else 0

                qk(0)
                for k in range(ntr):
                    if k + 1 < ntr:
                        qk(k + 1)
                    if nq is not None and qoff <= k < qoff + CT:
                        qproj_tile(c + 1, nq, k - qoff)
                    elif fin and k >= qoff + CT:
                        fin.pop(0)()
                    stp = st_tiles.pop(k)
                    et = ep.tile([P, SUB], bf16, tag="et", name="et")
                    nc.scalar.activation(et[:], stp[:], AF.Exp)
                    if k >= 4 * c:
                        nc.vector.tensor_mul(et[:], et[:],
                                             mask_sb[:, ts(k - 4 * c, SUB)])
                    if k == 0:
                        nc.vector.tensor_copy(acc[:], et[:])
                    else:
                        nc.vector.tensor_add(acc[:], acc[:], et[:])
                    ko, kj = divmod(k, 4)
                    ubase = ko * KCH + kj * 512
                    for cs in range(CT):
                        nc.tensor.matmul(
                            ht[cs][:],
                            xT_sb[:, ubase + cs * P: ubase + cs * P + P],
                            et[:], start=(k == 0), stop=(k == ntr - 1))
                nc.tensor.matmul(sm[:], ones_sb[:], acc[:],
                                 start=True, stop=True)
                if nq is not None:
                    chunk_q[c + 1] = nq
                while fin:
                    fin.pop(0)()
                return ht, sm

            def finish_thunks(c, ht, sm, last=False):
                hs = hp.tile([P, CT, SUB], bf16, tag="hs", name="hs")
                r_sb = op.tile([P, SUB], f32r, tag="rsb", name="r_sb")

                def copy_h():
                    # mid-pipeline ACT is busy with the next chunk's exps,
                    # so the copies run on DVE; for the final chunk both
                    # engines are idle and splitting halves the latency
                    for cs in range(CT):
                        if last and cs >= 2:
                            nc.scalar.activation(hs[:, cs, :], ht[cs][:],
                                                 AF.Identity)
                        else:
                            nc.vector.tensor_copy(hs[:, cs, :], ht[cs][:])

                def recip():
                    # row sums were accumulated broadcast across all
                    # partitions, so the reciprocal runs 128-way parallel and
                    # the result multiplies the output projection directly.
                    # Deferred off the chunk boundary so the copy_h burst is
                    # not delayed behind it on DVE.
                    with nc.allow_low_precision(reason="f32r is fp32-width"):
                        nc.vector.reciprocal(r_sb[:], sm[:])

                def proj_o(o):
                    pu = pp.tile([P, SUB], f32, tag="mm", name="pu")
                    for cj in range(CT):
                        nc.tensor.matmul(pu[:], wp_sb[:, cj, ts(o, P)],
                                         hs[:, cj, :],
                                         start=(cj == 0), stop=(cj == CT - 1))
                    og = op.tile([P, SUB], f32, tag="og", name="og")
                    nc.vector.tensor_mul(og[:], pu[:], r_sb[:])
                    # for the final chunk both DMA rings are idle: split the
                    # stores so the tail drains twice as fast
                    eng = nc.sync if (last and o % 2) else nc.scalar
                    eng.dma_start(out[o][:, ts(c, SUB)], og[:])

                return [copy_h, recip] + \
                    [lambda o=o: proj_o(o) for o in range(CT)]

            qproj(0)
            fin = []
            for c in range(NCH):
                ht, sm = s_loop(c, fin)
                fin = finish_thunks(c, ht, sm, last=(c == NCH - 1))
            while fin:
                fin.pop(0)()

    nc.finalize()
    return nc


def _masks(h):
    m = np.zeros((4, P, SUB), np.float32)
    f = np.arange(SUB)[None, :]
    p = np.arange(P)[:, None]
    m[0] = (f >= p).astype(np.float32)
    m[1] = (f >= p + 128).astype(np.float32)
    if h == 1:
        m[2] = 1.0
        m[3] = 1.0
    return m


def _pmajor(w):
    # [C_out, C_in] weight (transposed use) -> [P, CT, C] partition-major
    return np.ascontiguousarray(
        w.T.reshape(CT, P, C).transpose(1, 0, 2))


def _in_maps(inputs):
    x = np.asarray(inputs["x"], np.float32)
    Wq = np.asarray(inputs["Wq"], np.float64)
    bq = np.asarray(inputs["bq"], np.float64)
    Wk = np.asarray(inputs["Wk"], np.float64)
    Wv = np.asarray(inputs["Wv"], np.float64)
    bv = np.asarray(inputs["bv"], np.float64)
    Wp = np.asarray(inputs["Wp"], np.float64)
    bp = np.asarray(inputs["bp"], np.float64)

    Wt = (Wk.T @ Wq) * SCALE           # folded Q~ weights (scale included)
    bt = (Wk.T @ bq) * SCALE           # folded Q~ bias
    W2 = Wp @ Wv                       # folded output projection
    b2 = (bp + Wp @ bv).astype(np.float32)   # host-side constant bias

    common = {
        "wqd": _pmajor(Wt.astype(np.float32)).astype(ml_dtypes.bfloat16),
        "wpd": _pmajor(W2.astype(np.float32)).astype(ml_dtypes.bfloat16),
        "scd": np.ascontiguousarray(
            bt.astype(np.float32).reshape(CT, P).T),
        "oned": np.ones((P, P), np.float32),
    }
    maps = []
    for core in range(NCORE):
        b, h = divmod(core, 2)
        # per-512-block permutation: this core's query half first
        perm = (np.arange(NCH)[:, None] * 512
                + (h * SUB + np.arange(512)[None, :]) % 512).ravel()
        cols = (np.arange(NCH)[:, None] * 512 + h * SUB
                + np.arange(SUB)[None, :]).ravel()
        xp = x[b][:, perm].astype(ml_dtypes.bfloat16)     # [C, T]
        m = dict(common)
        # [chan, t] layout, chunk-outer: xkd[sc][p, cj*512 + t'] =
        #   xp[cj*128+p, sc*512+t']
        m["xkd"] = np.ascontiguousarray(
            xp.reshape(CT, P, NCH, 512).transpose(2, 1, 0, 3)
            .reshape(NCH, P, KCH))
        # [t, chan] layout, chunk-outer: xtd[sc][p, j*C + c] =
        #   xp[c, sc*512 + j*128 + p]
        m["xtd"] = np.ascontiguousarray(
            xp.T.reshape(NCH, 4, P, C).transpose(0, 2, 1, 3)
            .reshape(NCH, P, KCH))
        m["mkd"] = np.ascontiguousarray(
            _masks(h).transpose(1, 0, 2).reshape(P, 4 * SUB)
            .astype(ml_dtypes.bfloat16))
        maps.append((m, b, cols))
    return maps, b2


_prog_cache = {}


def _get_program():
    if "nc" not in _prog_cache:
        _prog_cache["nc"] = _build_program()
    return _prog_cache["nc"]


def kernel(**inputs):
    return _run(inputs, trace=False)[0]


def _run(inputs, trace=False):
    nc = _get_program()
    maps, b2 = _in_maps(inputs)
    res = run_bass_kernel_spmd(nc, [m for m, _, _ in maps],
                               core_ids=list(range(NCORE)), trace=trace)
    x = np.asarray(inputs["x"], np.float32)
    full = np.empty((B, C, T), np.float32)
    for core, (_, b, cols) in enumerate(maps):
        full[b][:, cols] = res.results[core]["out"].reshape(C, TQ)
    # residual + folded constant bias, both in full f32 on the host
    full += x + b2[None, :, None]
    return full, res


# revision 20
# speedup vs baseline: 1.5095x; 1.0063x over previous
"""Causal single-head attention 1D (B=4, C=512, T=4096) on 8 TRN2 NeuronCores.

Sharding: data-parallel over (batch, query-half). Each of the 8 cores handles
one batch b = core//2 and one query-half h = core%2. Host-side, each core's
copy of x[b] has every 512-wide block permuted so that the core's 256 query
columns sit FIRST within the block; the program is identical on all cores.

Algebraic folding (all host-side, exact):
  S[s,t] = (Wk x_s + bk).(Wq x_t + bq) = x_s.(W~ x_t + b~) + f(t), where
  W~ = Wk^T Wq, b~ = Wk^T bq, and f(t) is constant over keys s, so it cancels
  in the causal softmax.  Hence K == raw x (no K-projection) and a single
  Q~-projection with host-precomputed W~ (the 1/sqrt(C) scale folded in).
  Likewise h = E^T V with V = Wv x + bv gives
  Wp h = (Wp Wv)(x E) + (Wp bv) * sum(E), and sum(E) * (1/sum(E)) = 1, so
  raw x^T replaces V (no V-projection), the out-projection uses W2 = Wp Wv,
  and bias2 = bp + Wp bv is a constant added on the HOST after gather (the
  residual x is also added on the host, in full f32 precision).

Per core the device program is a single software-pipelined chunk loop:
  x streams in bf16 in two layouts ([chan, t] for K/Q~-moving, [t, chan] for
  the U matmul) straight into resident SBUF; per 256-query chunk: Q~ = W~ x,
  S = K-tiles^T Q~, E = exp(S) (bf16, causal-masked on the diagonal tiles),
  U += xT-tiles^T E and the denominator row-sums accumulate via an all-ones
  matmul broadcast over all 128 partitions (so the reciprocal runs fully
  parallel on DVE).  The next chunk's Q~-projection and the previous chunk's
  epilogue (U PSUM->SBUF copies, W2-projection, normalize, store) are
  dribbled into the key-tile loop so the PE never waits on the ACT/DVE
  chains.  Output stores ride the scalar DMA ring (free after the weights),
  the x streams ride the sync ring, all prefetched from the prologue.
"""

import numpy as np
import ml_dtypes

import concourse.bass as bass
import concourse.bacc as bacc
import concourse.mybir as mybir
from concourse import tile
from concourse.bass_utils import run_bass_kernel_spmd
from contextlib import ExitStack

B, C, T = 4, 512, 4096
NCORE = 8
P = 128
CT = C // P            # 4 channel tiles
NCH = T // 512         # 8 query chunks of 512
SUB = 256              # per-core queries per chunk
TQ = NCH * SUB         # 2048 queries per core
NST = T // P           # 32 key tiles
SCALE = float(C) ** -0.5
KCH = CT * 512         # bf16 elements per partition per x chunk (both layouts)

f32 = mybir.dt.float32
f32r = mybir.dt.float32r
bf16 = mybir.dt.bfloat16
AF = mybir.ActivationFunctionType
ts = bass.ts


def _build_program():
    nc = bacc.Bacc("TRN2", target_bir_lowering=False, debug=False,
                   num_devices=NCORE)

    # chunk-outer DRAM layouts so every chunk DMA is contiguous per partition
    xkd = nc.dram_tensor("xkd", [NCH, P, CT, 512], bf16,
                         kind="ExternalInput")
    xtd = nc.dram_tensor("xtd", [NCH, P, KCH], bf16, kind="ExternalInput")
    wqd = nc.dram_tensor("wqd", [P, CT, C], bf16, kind="ExternalInput")
    wpd = nc.dram_tensor("wpd", [P, CT, C], bf16, kind="ExternalInput")
    scd = nc.dram_tensor("scd", [P, CT], f32, kind="ExternalInput")
    mkd = nc.dram_tensor("mkd", [P, 4 * SUB], bf16, kind="ExternalInput")
    oned = nc.dram_tensor("oned", [P, P], f32, kind="ExternalInput")
    out = nc.dram_tensor("out", [CT, P, TQ], f32, kind="ExternalOutput")

    with tile.TileContext(nc) as tc, ExitStack() as ctx:
        const = ctx.enter_context(tc.tile_pool(name="const", bufs=1))

        k_sb = const.tile([P, NCH * KCH], bf16, tag="k")       # x, [chan, t]
        xT_sb = const.tile([P, NCH * KCH], bf16, tag="xt")     # x, [t, chan]
        wq_sb = const.tile([P, CT, C], bf16, tag="wq")
        wp_sb = const.tile([P, CT, C], bf16, tag="wp")
        mask_sb = const.tile([P, 4 * SUB], bf16, tag="mask")
        sc_sb = const.tile([P, CT], f32, tag="scs")
        ones_sb = const.tile([P, P], f32r, tag="ones")

        bq_sb = sc_sb  # b~ (adjusted Q bias) only

        # prologue DMAs, ordered by first use: the chunk-0 critical path
        # needs W~ (split across both rings) and chunk 0's QUERY columns of
        # K, so those 4 small slices go ahead of the rest of K0.  The sync
        # ring then carries the K stream, W2 and the later xT chunks in
        # consumption order; the scalar ring carries the small constants and
        # the first xT chunks (chunk 0 split per key tile so U(chunk 0)
        # pipelines with it), and later half the output stores.
        nc.sync.dma_start(wq_sb[:, 0:2, :], wqd[:][:, 0:2, :])
        nc.scalar.dma_start(wq_sb[:, 2:4, :], wqd[:][:, 2:4, :])
        nc.scalar.dma_start(sc_sb[:], scd[:])
        nc.scalar.dma_start(mask_sb[:], mkd[:])
        nc.scalar.dma_start(ones_sb[:], oned[:].bitcast(f32r))
        for cj in range(CT):
            nc.sync.dma_start(k_sb[:, cj * 512: cj * 512 + SUB],
                              xkd[0][:, cj, 0:SUB])
        for cj in range(CT):
            nc.sync.dma_start(k_sb[:, cj * 512 + SUB: (cj + 1) * 512],
                              xkd[0][:, cj, SUB:512])
        for j in range(4):
            nc.scalar.dma_start(xT_sb[:, ts(j, 512)], xtd[0][:, ts(j, 512)])
        nc.sync.dma_start(k_sb[:, KCH:2 * KCH], xkd[1][:, :, :])
        nc.scalar.dma_start(xT_sb[:, ts(1, KCH)], xtd[1])
        nc.sync.dma_start(k_sb[:, 2 * KCH:3 * KCH], xkd[2][:, :, :])
        nc.scalar.dma_start(xT_sb[:, ts(2, KCH)], xtd[2])
        nc.sync.dma_start(wp_sb[:], wpd[:])
        for sc in range(3, NCH):
            nc.sync.dma_start(k_sb[:, ts(sc, KCH)], xkd[sc][:, :, :])
            nc.sync.dma_start(xT_sb[:, ts(sc, KCH)], xtd[sc])

        pp = ctx.enter_context(tc.tile_pool(name="pp", bufs=3, space="PSUM"))
        ph = ctx.enter_context(tc.tile_pool(name="ph", bufs=1, space="PSUM"))

        with tc.tile_pool(name="qp", bufs=2) as qp, \
             tc.tile_pool(name="ep", bufs=4) as ep, \
             tc.tile_pool(name="ap", bufs=2) as ap, \
             tc.tile_pool(name="hp", bufs=2) as hp, \
             tc.tile_pool(name="op", bufs=5) as op:

            chunk_q = {}

            def qproj_tile(c, q_sb, o):
                pq = pp.tile([P, SUB], f32, tag="mm", name="pq")
                for cj in range(CT):
                    mv = k_sb[:, c * KCH + cj * 512: c * KCH + cj * 512 + SUB]
                    nc.tensor.matmul(
                        pq[:], wq_sb[:, cj, ts(o, P)], mv,
                        start=(cj == 0), stop=(cj == CT - 1))
                nc.scalar.activation(q_sb[:, o, :], pq[:], AF.Identity,
                                     bias=bq_sb[:, o:o + 1])

            def qproj(c):
                q_sb = qp.tile([P, CT, SUB], bf16, tag="q", name="q_sb")
                for o in range(CT):
                    qproj_tile(c, q_sb, o)
                chunk_q[c] = q_sb

            def s_loop(c, fin):
                """fin: list of deferred epilogue thunks for chunk c-1
                (h-copy burst first, then per-o projection+store), dribbled
                into this chunk's key-tile loop."""
                q_sb = chunk_q.pop(c)
                ntr = 4 * c + 4
                # U PSUM->SBUF copies of the previous chunk go first (DVE is
                # idle here)
                if fin:
                    fin.pop(0)()
                # separate PSUM tiles per accumulation group: the PSUM
                # pending-zero state from a matmul's start flag is tracked
                # per tensor/bank, so interleaved groups can't share a tile
                ht = [ph.tile([P, SUB], f32, tag=f"ht{cs}", name=f"ht{cs}")
                      for cs in range(CT)]
                sm = ph.tile([P, SUB], f32, tag="sm", name="sm")
                # denominator: E tiles accumulate on DVE (off the PE), one
                # broadcast ones-matmul per chunk turns the per-key partial
                # sums into the 128-partition-replicated row sum
                acc = ap.tile([P, SUB], f32r, tag="acc", name="acc")
                st_tiles = {}

                def qk(kk):
                    stp = pp.tile([P, SUB], f32, tag="mm", name="stp")
                    ko, kj = divmod(kk, 4)
                    base = ko * KCH + kj * P
                    for cj in range(CT):
                        nc.tensor.matmul(
                            stp[:], k_sb[:, base + cj * 512: base + cj * 512 + P],
                            q_sb[:, cj, :],
                            start=(cj == 0), stop=(cj == CT - 1))
                    st_tiles[kk] = stp

                # the NEXT chunk's Q projection is dribbled one output tile
                # per key-tile iteration: its PSUM slots then recycle at the
                # exp pace and its ACT copies stay off the chunk epilogue
                nq = qp.tile([P, CT, SUB], bf16, tag="q", name="q_sb") \
                    if c + 1 < NCH else None
                qoff = 1 if ntr > 4 else 0

                qk(0)
                for k in range(ntr):
                    if k + 1 < ntr:
                        qk(k + 1)
                    if nq is not None and qoff <= k < qoff + CT:
                        qproj_tile(c + 1, nq, k - qoff)
                    elif fin and k >= qoff + CT:
                        fin.pop(0)()
                    stp = st_tiles.pop(k)
                    et = ep.tile([P, SUB], bf16, tag="et", name="et")
                    nc.scalar.activation(et[:], stp[:], AF.Exp)
                    if k >= 4 * c:
                        nc.vector.tensor_mul(et[:], et[:],
                                             mask_sb[:, ts(k - 4 * c, SUB)])
                    if k == 0:
                        nc.vector.tensor_copy(acc[:], et[:])
                    else:
                        nc.vector.tensor_add(acc[:], acc[:], et[:])
                    ko, kj = divmod(k, 4)
                    ubase = ko * KCH + kj * 512
                    for cs in range(CT):
                        nc.tensor.matmul(
                            ht[cs][:],
                            xT_sb[:, ubase + cs * P: ubase + cs * P + P],
                            et[:], start=(k == 0), stop=(k == ntr - 1))
                nc.tensor.matmul(sm[:], ones_sb[:], acc[:],
                                 start=True, stop=True)
                if nq is not None:
                    chunk_q[c + 1] = nq
                while fin:
                    fin.pop(0)()
                return ht, sm

            def finish_thunks(c, ht, sm, last=False):
                hs = hp.tile([P, CT, SUB], bf16, tag="hs", name="hs")
                r_sb = op.tile([P, SUB], f32r, tag="rsb", name="r_sb")

                def copy_h():
                    # mid-pipeline ACT is busy with the next chunk's exps,
                    # so the copies run on DVE; for the final chunk both
                    # engines are idle and splitting halves the latency
                    for cs in range(CT):
                        if last and cs >= 2:
                            nc.scalar.activation(hs[:, cs, :], ht[cs][:],
                                                 AF.Identity)
                        else:
                            nc.vector.tensor_copy(hs[:, cs, :], ht[cs][:])

                def recip():
                    # row sums were accumulated broadcast across all
                    # partitions, so the reciprocal runs 128-way parallel and
                    # the result multiplies the output projection directly.
                    # Deferred off the chunk boundary so the copy_h burst is
                    # not delayed behind it on DVE.
                    with nc.allow_low_precision(reason="f32r is fp32-width"):
                        nc.vector.reciprocal(r_sb[:], sm[:])

                def proj_o(o):
                    pu = pp.tile([P, SUB], f32, tag="mm", name="pu")
                    for cj in range(CT):
                        nc.tensor.matmul(pu[:], wp_sb[:, cj, ts(o, P)],
                                         hs[:, cj, :],
                                         start=(cj == 0), stop=(cj == CT - 1))
                    og = op.tile([P, SUB], f32, tag="og", name="og")
                    nc.vector.tensor_mul(og[:], pu[:], r_sb[:])
                    # for the final chunk both DMA rings are idle: split the
                    # stores so the tail drains twice as fast
                    eng = nc.sync if (last and o % 2) else nc.scalar
                    eng.dma_start(out[o][:, ts(c, SUB)], og[:])

                return [copy_h, recip] + \
                    [lambda o=o: proj_o(o) for o in range(CT)]

            qproj(0)
            fin = []
            for c in range(NCH):
                ht, sm = s_loop(c, fin)
                fin = finish_thunks(c, ht, sm, last=(c == NCH - 1))
            while fin:
                fin.pop(0)()

    nc.finalize()
    return nc


def _masks(h):
    m = np.zeros((4, P, SUB), np.float32)
    f = np.arange(SUB)[None, :]
    p = np.arange(P)[:, None]
    m[0] = (f >= p).astype(np.float32)
    m[1] = (f >= p + 128).astype(np.float32)
    if h == 1:
        m[2] = 1.0
        m[3] = 1.0
    return m


def _pmajor(w):
    # [C_out, C_in] weight (transposed use) -> [P, CT, C] partition-major
    return np.ascontiguousarray(
        w.T.reshape(CT, P, C).transpose(1, 0, 2))


def _in_maps(inputs):
    x = np.asarray(inputs["x"], np.float32)
    Wq = np.asarray(inputs["Wq"], np.float64)
    bq = np.asarray(inputs["bq"], np.float64)
    Wk = np.asarray(inputs["Wk"], np.float64)
    Wv = np.asarray(inputs["Wv"], np.float64)
    bv = np.asarray(inputs["bv"], np.float64)
    Wp = np.asarray(inputs["Wp"], np.float64)
    bp = np.asarray(inputs["bp"], np.float64)

    Wt = (Wk.T @ Wq) * SCALE           # folded Q~ weights (scale included)
    bt = (Wk.T @ bq) * SCALE           # folded Q~ bias
    W2 = Wp @ Wv                       # folded output projection
    b2 = (bp + Wp @ bv).astype(np.float32)   # host-side constant bias

    common = {
        "wqd": _pmajor(Wt.astype(np.float32)).astype(ml_dtypes.bfloat16),
        "wpd": _pmajor(W2.astype(np.float32)).astype(ml_dtypes.bfloat16),
        "scd": np.ascontiguousarray(
            bt.astype(np.float32).reshape(CT, P).T),
        "oned": np.ones((P, P), np.float32),
    }
    maps = []
    for core in range(NCORE):
        b, h = divmod(core, 2)
        # per-512-block permutation: this core's query half first
        perm = (np.arange(NCH)[:, None] * 512
                + (h * SUB + np.arange(512)[None, :]) % 512).ravel()
        cols = (np.arange(NCH)[:, None] * 512 + h * SUB
                + np.arange(SUB)[None, :]).ravel()
        xp = x[b][:, perm].astype(ml_dtypes.bfloat16)     # [C, T]
        m = dict(common)
        # [chan, t] layout, chunk-outer: xkd[sc][p, cj*512 + t'] =
        #   xp[cj*128+p, sc*512+t']
        m["xkd"] = np.ascontiguousarray(
            xp.reshape(CT, P, NCH, 512).transpose(2, 1, 0, 3))
        # [t, chan] layout, chunk-outer: xtd[sc][p, j*C + c] =
        #   xp[c, sc*512 + j*128 + p]
        m["xtd"] = np.ascontiguousarray(
            xp.T.reshape(NCH, 4, P, C).transpose(0, 2, 1, 3)
            .reshape(NCH, P, KCH))
        m["mkd"] = np.ascontiguousarray(
            _masks(h).transpose(1, 0, 2).reshape(P, 4 * SUB)
            .astype(ml_dtypes.bfloat16))
        maps.append((m, b, cols))
    return maps, b2


_prog_cache = {}


def _get_program():
    if "nc" not in _prog_cache:
        _prog_cache["nc"] = _build_program()
    return _prog_cache["nc"]


def kernel(**inputs):
    return _run(inputs, trace=False)[0]


def _run(inputs, trace=False):
    nc = _get_program()
    maps, b2 = _in_maps(inputs)
    res = run_bass_kernel_spmd(nc, [m for m, _, _ in maps],
                               core_ids=list(range(NCORE)), trace=trace)
    x = np.asarray(inputs["x"], np.float32)
    full = np.empty((B, C, T), np.float32)
    for core, (_, b, cols) in enumerate(maps):
        full[b][:, cols] = res.results[core]["out"].reshape(C, TQ)
    # residual + folded constant bias, both in full f32 on the host
    full += x + b2[None, :, None]
    return full, res


# revision 24
# speedup vs baseline: 1.5270x; 1.0116x over previous
"""Causal single-head attention 1D (B=4, C=512, T=4096) on 8 TRN2 NeuronCores.

Sharding: data-parallel over (batch, query-half). Each of the 8 cores handles
one batch b = core//2 and one query-half h = core%2. Host-side, each core's
copy of x[b] has every 512-wide block permuted so that the core's 256 query
columns sit FIRST within the block; the program is identical on all cores.

Algebraic folding (all host-side, exact):
  S[s,t] = (Wk x_s + bk).(Wq x_t + bq) = x_s.(W~ x_t + b~) + f(t), where
  W~ = Wk^T Wq, b~ = Wk^T bq, and f(t) is constant over keys s, so it cancels
  in the causal softmax.  Hence K == raw x (no K-projection) and a single
  Q~-projection with host-precomputed W~ (the 1/sqrt(C) scale folded in).
  Likewise h = E^T V with V = Wv x + bv gives
  Wp h = (Wp Wv)(x E) + (Wp bv) * sum(E), and sum(E) * (1/sum(E)) = 1, so
  raw x^T replaces V (no V-projection), the out-projection uses W2 = Wp Wv,
  and bias2 = bp + Wp bv is a constant added on the HOST after gather (the
  residual x is also added on the host, in full f32 precision).

Per core the device program is a single software-pipelined chunk loop:
  x streams in bf16 in two layouts ([chan, t] for K/Q~-moving, [t, chan] for
  the U matmul) straight into resident SBUF; per 256-query chunk: Q~ = W~ x,
  S = K-tiles^T Q~, E = exp(S) (bf16, causal-masked on the diagonal tiles),
  U += xT-tiles^T E and the denominator row-sums accumulate via an all-ones
  matmul broadcast over all 128 partitions (so the reciprocal runs fully
  parallel on DVE).  The next chunk's Q~-projection and the previous chunk's
  epilogue (U PSUM->SBUF copies, W2-projection, normalize, store) are
  dribbled into the key-tile loop so the PE never waits on the ACT/DVE
  chains.  Output stores ride the scalar DMA ring (free after the weights),
  the x streams ride the sync ring, all prefetched from the prologue.
"""

import numpy as np
import ml_dtypes

import concourse.bass as bass
import concourse.bacc as bacc
import concourse.mybir as mybir
from concourse import tile
from concourse.bass_utils import run_bass_kernel_spmd
from contextlib import ExitStack

B, C, T = 4, 512, 4096
NCORE = 8
P = 128
CT = C // P            # 4 channel tiles
NCH = T // 512         # 8 query chunks of 512
SUB = 256              # per-core queries per chunk
TQ = NCH * SUB         # 2048 queries per core
NST = T // P           # 32 key tiles
SCALE = float(C) ** -0.5
KCH = CT * 512         # bf16 elements per partition per x chunk (both layouts)

f32 = mybir.dt.float32
f32r = mybir.dt.float32r
bf16 = mybir.dt.bfloat16
AF = mybir.ActivationFunctionType
ts = bass.ts


def _build_program():
    nc = bacc.Bacc("TRN2", target_bir_lowering=False, debug=False,
                   num_devices=NCORE)

    # chunk-outer DRAM layouts so every chunk DMA is contiguous per partition
    xkd = nc.dram_tensor("xkd", [NCH, P, CT, 512], bf16,
                         kind="ExternalInput")
    xtd = nc.dram_tensor("xtd", [NCH, P, KCH], bf16, kind="ExternalInput")
    wqd = nc.dram_tensor("wqd", [P, CT, C], bf16, kind="ExternalInput")
    wpd = nc.dram_tensor("wpd", [P, CT, C], bf16, kind="ExternalInput")
    scd = nc.dram_tensor("scd", [P, CT], f32, kind="ExternalInput")
    mkd = nc.dram_tensor("mkd", [P, 4 * SUB], bf16, kind="ExternalInput")
    oned = nc.dram_tensor("oned", [P, P], f32, kind="ExternalInput")
    out = nc.dram_tensor("out", [CT, P, TQ], f32, kind="ExternalOutput")

    with tile.TileContext(nc) as tc, ExitStack() as ctx:
        const = ctx.enter_context(tc.tile_pool(name="const", bufs=1))

        k_sb = const.tile([P, NCH * KCH], bf16, tag="k")       # x, [chan, t]
        xT_sb = const.tile([P, NCH * KCH], bf16, tag="xt")     # x, [t, chan]
        wq_sb = const.tile([P, CT, C], bf16, tag="wq")
        wp_sb = const.tile([P, CT, C], bf16, tag="wp")
        mask_sb = const.tile([P, 4 * SUB], bf16, tag="mask")
        sc_sb = const.tile([P, CT], f32, tag="scs")
        ones_sb = const.tile([P, P], f32r, tag="ones")

        bq_sb = sc_sb  # b~ (adjusted Q bias) only

        # prologue DMAs, ordered by first use, whole chunks per transfer
        # (4 KB per-partition rows -- splitting these into finer slices
        # measurably degrades early DMA-ring throughput).  The first ~20us
        # is DMA-bound, so the two rings carry the early chunks balanced by
        # need time; the scalar ring later carries half the output stores.
        nc.sync.dma_start(wq_sb[:, 0:2, :], wqd[:][:, 0:2, :])
        nc.scalar.dma_start(wq_sb[:, 2:4, :], wqd[:][:, 2:4, :])
        nc.scalar.dma_start(sc_sb[:], scd[:])
        nc.scalar.dma_start(mask_sb[:], mkd[:])
        nc.scalar.dma_start(ones_sb[:], oned[:].bitcast(f32r))
        nc.sync.dma_start(k_sb[:, 0:KCH], xkd[0][:, :, :])
        nc.scalar.dma_start(xT_sb[:, 0:KCH], xtd[0])
        nc.sync.dma_start(k_sb[:, KCH:2 * KCH], xkd[1][:, :, :])
        nc.scalar.dma_start(xT_sb[:, ts(1, KCH)], xtd[1])
        nc.sync.dma_start(k_sb[:, 2 * KCH:3 * KCH], xkd[2][:, :, :])
        nc.scalar.dma_start(xT_sb[:, ts(2, KCH)], xtd[2])
        nc.sync.dma_start(k_sb[:, 3 * KCH:4 * KCH], xkd[3][:, :, :])
        nc.scalar.dma_start(wp_sb[:], wpd[:])
        nc.sync.dma_start(xT_sb[:, ts(3, KCH)], xtd[3])
        for sc in range(4, NCH):
            nc.sync.dma_start(k_sb[:, ts(sc, KCH)], xkd[sc][:, :, :])
            nc.sync.dma_start(xT_sb[:, ts(sc, KCH)], xtd[sc])

        pp = ctx.enter_context(tc.tile_pool(name="pp", bufs=3, space="PSUM"))
        ph = ctx.enter_context(tc.tile_pool(name="ph", bufs=1, space="PSUM"))

        with tc.tile_pool(name="qp", bufs=2) as qp, \
             tc.tile_pool(name="ep", bufs=4) as ep, \
             tc.tile_pool(name="ap", bufs=2) as ap, \
             tc.tile_pool(name="hp", bufs=2) as hp, \
             tc.tile_pool(name="op", bufs=5) as op:

            chunk_q = {}

            # PE warm-up: the HAM clock gate keeps the PE at 1.2 GHz until
            # it has seen ~3.4us of sustained matmul activity.  The first
            # real matmul can't start until W~ and K0 land (~13us), so burn
            # the DMA wait on dummy matmuls (reading the just-arrived W~
            # halves as garbage operands) to flip the gate to 2.4 GHz before
            # chunk 0 begins.
            def warmup():
                wt = pp.tile([P, 64], f32, tag="mm", name="warm")
                for i in range(24):
                    nc.tensor.matmul(wt[:], wq_sb[:, 0, 0:P],
                                     wq_sb[:, 1, 0:64],
                                     start=True, stop=True,
                                     skip_group_check=True)

            def qproj_tile(c, q_sb, o):
                pq = pp.tile([P, SUB], f32, tag="mm", name="pq")
                for cj in range(CT):
                    mv = k_sb[:, c * KCH + cj * 512: c * KCH + cj * 512 + SUB]
                    nc.tensor.matmul(
                        pq[:], wq_sb[:, cj, ts(o, P)], mv,
                        start=(cj == 0), stop=(cj == CT - 1))
                nc.scalar.activation(q_sb[:, o, :], pq[:], AF.Identity,
                                     bias=bq_sb[:, o:o + 1])

            def qproj(c):
                q_sb = qp.tile([P, CT, SUB], bf16, tag="q", name="q_sb")
                for o in range(CT):
                    qproj_tile(c, q_sb, o)
                chunk_q[c] = q_sb

            def s_loop(c, fin):
                """fin: list of deferred epilogue thunks for chunk c-1
                (h-copy burst first, then per-o projection+store), dribbled
                into this chunk's key-tile loop."""
                q_sb = chunk_q.pop(c)
                ntr = 4 * c + 4
                # U PSUM->SBUF copies of the previous chunk go first (DVE is
                # idle here)
                if fin:
                    fin.pop(0)()
                # separate PSUM tiles per accumulation group: the PSUM
                # pending-zero state from a matmul's start flag is tracked
                # per tensor/bank, so interleaved groups can't share a tile
                ht = [ph.tile([P, SUB], f32, tag=f"ht{cs}", name=f"ht{cs}")
                      for cs in range(CT)]
                sm = ph.tile([P, SUB], f32, tag="sm", name="sm")
                # denominator: E tiles accumulate on DVE (off the PE), one
                # broadcast ones-matmul per chunk turns the per-key partial
                # sums into the 128-partition-replicated row sum
                acc = ap.tile([P, SUB], f32r, tag="acc", name="acc")
                st_tiles = {}

                def qk(kk):
                    stp = pp.tile([P, SUB], f32, tag="mm", name="stp")
                    ko, kj = divmod(kk, 4)
                    base = ko * KCH + kj * P
                    for cj in range(CT):
                        nc.tensor.matmul(
                            stp[:], k_sb[:, base + cj * 512: base + cj * 512 + P],
                            q_sb[:, cj, :],
                            start=(cj == 0), stop=(cj == CT - 1))
                    st_tiles[kk] = stp

                # the NEXT chunk's Q projection is dribbled one output tile
                # per key-tile iteration: its PSUM slots then recycle at the
                # exp pace and its ACT copies stay off the chunk epilogue
                nq = qp.tile([P, CT, SUB], bf16, tag="q", name="q_sb") \
                    if c + 1 < NCH else None
                qoff = 1 if ntr > 4 else 0

                qk(0)
                for k in range(ntr):
                    if k + 1 < ntr:
                        qk(k + 1)
                    if nq is not None and qoff <= k < qoff + CT:
                        qproj_tile(c + 1, nq, k - qoff)
                    elif fin and k >= qoff + CT:
                        fin.pop(0)()
                    stp = st_tiles.pop(k)
                    et = ep.tile([P, SUB], bf16, tag="et", name="et")
                    nc.scalar.activation(et[:], stp[:], AF.Exp)
                    if k >= 4 * c:
                        nc.vector.tensor_mul(et[:], et[:],
                                             mask_sb[:, ts(k - 4 * c, SUB)])
                    if k == 0:
                        nc.vector.tensor_copy(acc[:], et[:])
                    else:
                        nc.vector.tensor_add(acc[:], acc[:], et[:])
                    ko, kj = divmod(k, 4)
                    ubase = ko * KCH + kj * 512
                    for cs in range(CT):
                        nc.tensor.matmul(
                            ht[cs][:],
                            xT_sb[:, ubase + cs * P: ubase + cs * P + P],
                            et[:], start=(k == 0), stop=(k == ntr - 1))
                nc.tensor.matmul(sm[:], ones_sb[:], acc[:],
                                 start=True, stop=True)
                if nq is not None:
                    chunk_q[c + 1] = nq
                while fin:
                    fin.pop(0)()
                return ht, sm

            def finish_thunks(c, ht, sm, last=False):
                hs = hp.tile([P, CT, SUB], bf16, tag="hs", name="hs")
                r_sb = op.tile([P, SUB], f32r, tag="rsb", name="r_sb")

                def copy_h():
                    # mid-pipeline ACT is busy with the next chunk's exps,
                    # so the copies run on DVE; for the final chunk both
                    # engines are idle and splitting halves the latency
                    for cs in range(CT):
                        if last and cs >= 2:
                            nc.scalar.activation(hs[:, cs, :], ht[cs][:],
                                                 AF.Identity)
                        else:
                            nc.vector.tensor_copy(hs[:, cs, :], ht[cs][:])

                def recip():
                    # row sums were accumulated broadcast across all
                    # partitions, so the reciprocal runs 128-way parallel and
                    # the result multiplies the output projection directly.
                    # Deferred off the chunk boundary so the copy_h burst is
                    # not delayed behind it on DVE.
                    with nc.allow_low_precision(reason="f32r is fp32-width"):
                        nc.vector.reciprocal(r_sb[:], sm[:])

                def proj_o(o):
                    pu = pp.tile([P, SUB], f32, tag="mm", name="pu")
                    for cj in range(CT):
                        nc.tensor.matmul(pu[:], wp_sb[:, cj, ts(o, P)],
                                         hs[:, cj, :],
                                         start=(cj == 0), stop=(cj == CT - 1))
                    og = op.tile([P, SUB], f32, tag="og", name="og")
                    nc.vector.tensor_mul(og[:], pu[:], r_sb[:])
                    # for the final chunk both DMA rings are idle: split the
                    # stores so the tail drains twice as fast
                    eng = nc.sync if (last and o % 2) else nc.scalar
                    eng.dma_start(out[o][:, ts(c, SUB)], og[:])

                return [copy_h, recip] + \
                    [lambda o=o: proj_o(o) for o in range(CT)]

            warmup()
            qproj(0)
            fin = []
            for c in range(NCH):
                ht, sm = s_loop(c, fin)
                fin = finish_thunks(c, ht, sm, last=(c == NCH - 1))
            while fin:
                fin.pop(0)()

    nc.finalize()
    return nc


def _masks(h):
    m = np.zeros((4, P, SUB), np.float32)
    f = np.arange(SUB)[None, :]
    p = np.arange(P)[:, None]
    m[0] = (f >= p).astype(np.float32)
    m[1] = (f >= p + 128).astype(np.float32)
    if h == 1:
        m[2] = 1.0
        m[3] = 1.0
    return m


def _pmajor(w):
    # [C_out, C_in] weight (transposed use) -> [P, CT, C] partition-major
    return np.ascontiguousarray(
        w.T.reshape(CT, P, C).transpose(1, 0, 2))


def _in_maps(inputs):
    x = np.asarray(inputs["x"], np.float32)
    Wq = np.asarray(inputs["Wq"], np.float64)
    bq = np.asarray(inputs["bq"], np.float64)
    Wk = np.asarray(inputs["Wk"], np.float64)
    Wv = np.asarray(inputs["Wv"], np.float64)
    bv = np.asarray(inputs["bv"], np.float64)
    Wp = np.asarray(inputs["Wp"], np.float64)
    bp = np.asarray(inputs["bp"], np.float64)

    Wt = (Wk.T @ Wq) * SCALE           # folded Q~ weights (scale included)
    bt = (Wk.T @ bq) * SCALE           # folded Q~ bias
    W2 = Wp @ Wv                       # folded output projection
    b2 = (bp + Wp @ bv).astype(np.float32)   # host-side constant bias

    common = {
        "wqd": _pmajor(Wt.astype(np.float32)).astype(ml_dtypes.bfloat16),
        "wpd": _pmajor(W2.astype(np.float32)).astype(ml_dtypes.bfloat16),
        "scd": np.ascontiguousarray(
            bt.astype(np.float32).reshape(CT, P).T),
        "oned": np.ones((P, P), np.float32),
    }
    maps = []
    for core in range(NCORE):
        b, h = divmod(core, 2)
        # per-512-block permutation: this core's query half first
        perm = (np.arange(NCH)[:, None] * 512
                + (h * SUB + np.arange(512)[None, :]) % 512).ravel()
        cols = (np.arange(NCH)[:, None] * 512 + h * SUB
                + np.arange(SUB)[None, :]).ravel()
        xp = x[b][:, perm].astype(ml_dtypes.bfloat16)     # [C, T]
        m = dict(common)
        # [chan, t] layout, chunk-outer: xkd[sc][p, cj*512 + t'] =
        #   xp[cj*128+p, sc*512+t']
        m["xkd"] = np.ascontiguousarray(
            xp.reshape(CT, P, NCH, 512).transpose(2, 1, 0, 3))
        # [t, chan] layout, chunk-outer: xtd[sc][p, j*C + c] =
        #   xp[c, sc*512 + j*128 + p]
        m["xtd"] = np.ascontiguousarray(
            xp.T.reshape(NCH, 4, P, C).transpose(0, 2, 1, 3)
            .reshape(NCH, P, KCH))
        m["mkd"] = np.ascontiguousarray(
            _masks(h).transpose(1, 0, 2).reshape(P, 4 * SUB)
            .astype(ml_dtypes.bfloat16))
        maps.append((m, b, cols))
    return maps, b2


_prog_cache = {}


def _get_program():
    if "nc" not in _prog_cache:
        _prog_cache["nc"] = _build_program()
    return _prog_cache["nc"]


def kernel(**inputs):
    return _run(inputs, trace=False)[0]


def _run(inputs, trace=False):
    nc = _get_program()
    maps, b2 = _in_maps(inputs)
    res = run_bass_kernel_spmd(nc, [m for m, _, _ in maps],
                               core_ids=list(range(NCORE)), trace=trace)
    x = np.asarray(inputs["x"], np.float32)
    full = np.empty((B, C, T), np.float32)
    for core, (_, b, cols) in enumerate(maps):
        full[b][:, cols] = res.results[core]["out"].reshape(C, TQ)
    # residual + folded constant bias, both in full f32 on the host
    full += x + b2[None, :, None]
    return full, res


# revision 32
# speedup vs baseline: 1.5803x; 1.0349x over previous
"""Causal single-head attention 1D (B=4, C=512, T=4096) on 8 TRN2 NeuronCores.

Sharding: data-parallel over (batch, query-half). Each of the 8 cores handles
one batch b = core//2 and one query-half h = core%2. Host-side, each core's
copy of x[b] has every 512-wide block permuted so that the core's 256 query
columns sit FIRST within the block; the program is identical on all cores.

Algebraic folding (all host-side, exact):
  S[s,t] = (Wk x_s + bk).(Wq x_t + bq) = x_s.(W~ x_t + b~) + f(t), where
  W~ = Wk^T Wq, b~ = Wk^T bq, and f(t) is constant over keys s, so it cancels
  in the causal softmax.  Hence K == raw x (no K-projection) and a single
  Q~-projection with host-precomputed W~ (the 1/sqrt(C) scale folded in).
  Likewise h = E^T V with V = Wv x + bv gives
  Wp h = (Wp Wv)(x E) + (Wp bv) * sum(E), and sum(E) * (1/sum(E)) = 1, so
  raw x^T replaces V (no V-projection), the out-projection uses W2 = Wp Wv,
  and bias2 = bp + Wp bv is a constant added on the HOST after gather (the
  residual x is also added on the host, in full f32 precision).

Per core the device program is a single software-pipelined chunk loop:
  x streams in bf16 in two layouts ([chan, t] for K/Q~-moving, [t, chan] for
  the U matmul) straight into resident SBUF; per 256-query chunk: Q~ = W~ x,
  S = K-tiles^T Q~, E = exp(S) (bf16, causal-masked on the diagonal tiles),
  U += xT-tiles^T E and the denominator row-sums accumulate via an all-ones
  matmul broadcast over all 128 partitions (so the reciprocal runs fully
  parallel on DVE).  The next chunk's Q~-projection and the previous chunk's
  epilogue (U PSUM->SBUF copies, W2-projection, normalize, store) are
  dribbled into the key-tile loop so the PE never waits on the ACT/DVE
  chains.  Output stores ride the scalar DMA ring (free after the weights),
  the x streams ride the sync ring, all prefetched from the prologue.
"""

import numpy as np
import ml_dtypes

import concourse.bass as bass
import concourse.bacc as bacc
import concourse.mybir as mybir
from concourse import tile
from concourse.bass_utils import run_bass_kernel_spmd
from contextlib import ExitStack

B, C, T = 4, 512, 4096
NCORE = 8
P = 128
CT = C // P            # 4 channel tiles
NCH = T // 512         # 8 query chunks of 512
SUB = 256              # per-core queries per chunk
TQ = NCH * SUB         # 2048 queries per core
NST = T // P           # 32 key tiles
SCALE = float(C) ** -0.5
KCH = CT * 512         # bf16 elements per partition per x chunk (both layouts)

f32 = mybir.dt.float32
f32r = mybir.dt.float32r
bf16 = mybir.dt.bfloat16
AF = mybir.ActivationFunctionType
ts = bass.ts


def _build_program():
    nc = bacc.Bacc("TRN2", target_bir_lowering=False, debug=False,
                   num_devices=NCORE)

    # chunk-outer DRAM layouts so every chunk DMA is contiguous per partition
    xkd = nc.dram_tensor("xkd", [NCH, P, CT, 512], bf16,
                         kind="ExternalInput")
    xtd = nc.dram_tensor("xtd", [NCH, P, KCH], bf16, kind="ExternalInput")
    wqd = nc.dram_tensor("wqd", [P, CT, C], bf16, kind="ExternalInput")
    wpd = nc.dram_tensor("wpd", [P, CT, C], bf16, kind="ExternalInput")
    scd = nc.dram_tensor("scd", [P, CT], f32, kind="ExternalInput")
    mkd = nc.dram_tensor("mkd", [P, 4 * SUB], bf16, kind="ExternalInput")
    oned = nc.dram_tensor("oned", [P, P], f32, kind="ExternalInput")
    out = nc.dram_tensor("out", [P, CT, TQ], f32, kind="ExternalOutput")

    with tile.TileContext(nc) as tc, ExitStack() as ctx:
        const = ctx.enter_context(tc.tile_pool(name="const", bufs=1))

        k_sb = const.tile([P, NCH * KCH], bf16, tag="k")       # x, [chan, t]
        xT_sb = const.tile([P, NCH * KCH], bf16, tag="xt")     # x, [t, chan]
        wq_sb = const.tile([P, CT, C], bf16, tag="wq")
        wp_sb = const.tile([P, CT, C], bf16, tag="wp")
        mask_sb = const.tile([P, 4 * SUB], bf16, tag="mask")
        sc_sb = const.tile([P, CT], f32, tag="scs")
        ones_sb = const.tile([P, P], f32r, tag="ones")

        bq_sb = sc_sb  # b~ (adjusted Q bias) only

        # prologue DMAs, ordered by first use, whole chunks per transfer
        # (4 KB per-partition rows -- splitting these into finer slices
        # measurably degrades early DMA-ring throughput).  The first ~20us
        # is DMA-bound, so the two rings carry the early chunks balanced by
        # need time; the scalar ring later carries half the output stores.
        nc.sync.dma_start(wq_sb[:, 0:2, :], wqd[:][:, 0:2, :])
        nc.scalar.dma_start(wq_sb[:, 2:4, :], wqd[:][:, 2:4, :])
        nc.scalar.dma_start(sc_sb[:], scd[:])
        nc.scalar.dma_start(mask_sb[:], mkd[:])
        nc.scalar.dma_start(ones_sb[:], oned[:].bitcast(f32r))
        nc.scalar.dma_start(wp_sb[:], wpd[:])
        # both x streams ride the sync ring: the scalar hwdge ring is
        # measurably slower in the early DMA-bound phase
        for sc in range(NCH):
            nc.sync.dma_start(k_sb[:, ts(sc, KCH)], xkd[sc][:, :, :])
            nc.sync.dma_start(xT_sb[:, ts(sc, KCH)], xtd[sc])

        pp = ctx.enter_context(tc.tile_pool(name="pp", bufs=3, space="PSUM"))
        ph = ctx.enter_context(tc.tile_pool(name="ph", bufs=1, space="PSUM"))

        with tc.tile_pool(name="qp", bufs=2) as qp, \
             tc.tile_pool(name="ep", bufs=4) as ep, \
             tc.tile_pool(name="ap", bufs=2) as ap, \
             tc.tile_pool(name="hp", bufs=2) as hp, \
             tc.tile_pool(name="op", bufs=3) as op, \
             tc.tile_pool(name="ob", bufs=2) as ob:

            chunk_q = {}

            # PE warm-up: the HAM clock gate keeps the PE at 1.2 GHz until
            # it has seen ~3.4us of sustained matmul activity.  The first
            # real matmul can't start until W~ and K0 land (~13us), so burn
            # the DMA wait on dummy matmuls (reading the just-arrived W~
            # halves as garbage operands) to flip the gate to 2.4 GHz before
            # chunk 0 begins.
            def warmup():
                wt = pp.tile([P, 64], f32, tag="mm", name="warm")
                for i in range(24):
                    nc.tensor.matmul(wt[:], wq_sb[:, 0, 0:P],
                                     wq_sb[:, 1, 0:64],
                                     start=True, stop=True,
                                     skip_group_check=True)

            def qproj_tile(c, q_sb, o):
                pq = pp.tile([P, SUB], f32, tag="mm", name="pq")
                for cj in range(CT):
                    mv = k_sb[:, c * KCH + cj * 512: c * KCH + cj * 512 + SUB]
                    nc.tensor.matmul(
                        pq[:], wq_sb[:, cj, ts(o, P)], mv,
                        start=(cj == 0), stop=(cj == CT - 1))
                nc.scalar.activation(q_sb[:, o, :], pq[:], AF.Identity,
                                     bias=bq_sb[:, o:o + 1])

            def qproj(c):
                q_sb = qp.tile([P, CT, SUB], bf16, tag="q", name="q_sb")
                for o in range(CT):
                    qproj_tile(c, q_sb, o)
                chunk_q[c] = q_sb

            def s_loop(c, fin):
                """fin: list of deferred epilogue thunks for chunk c-1
                (h-copy burst first, then per-o projection+store), dribbled
                into this chunk's key-tile loop."""
                q_sb = chunk_q.pop(c)
                ntr = 4 * c + 4
                # U PSUM->SBUF copies of the previous chunk go first (DVE is
                # idle here)
                if fin:
                    fin.pop(0)()
                # separate PSUM tiles per accumulation group: the PSUM
                # pending-zero state from a matmul's start flag is tracked
                # per tensor/bank, so interleaved groups can't share a tile
                ht = [ph.tile([P, SUB], f32, tag=f"ht{cs}", name=f"ht{cs}")
                      for cs in range(CT)]
                sm = ph.tile([P, SUB], f32, tag="sm", name="sm")
                # denominator: E tiles accumulate on DVE (off the PE), one
                # broadcast ones-matmul per chunk turns the per-key partial
                # sums into the 128-partition-replicated row sum
                acc = ap.tile([P, SUB], f32r, tag="acc", name="acc")
                st_tiles = {}

                def qk(kk):
                    stp = pp.tile([P, SUB], f32, tag="mm", name="stp")
                    ko, kj = divmod(kk, 4)
                    base = ko * KCH + kj * P
                    for cj in range(CT):
                        nc.tensor.matmul(
                            stp[:], k_sb[:, base + cj * 512: base + cj * 512 + P],
                            q_sb[:, cj, :],
                            start=(cj == 0), stop=(cj == CT - 1))
                    st_tiles[kk] = stp

                # the NEXT chunk's Q projection is dribbled one output tile
                # per key-tile iteration: its PSUM slots then recycle at the
                # exp pace and its ACT copies stay off the chunk epilogue
                nq = qp.tile([P, CT, SUB], bf16, tag="q", name="q_sb") \
                    if c + 1 < NCH else None
                qoff = 1 if ntr > 4 else 0

                qk(0)
                for k in range(ntr):
                    if k + 1 < ntr:
                        qk(k + 1)
                    if nq is not None and c > 0 and qoff <= k < qoff + CT:
                        qproj_tile(c + 1, nq, k - qoff)
                    elif fin and k >= qoff + CT:
                        fin.pop(0)()
                    stp = st_tiles.pop(k)
                    et = ep.tile([P, SUB], bf16, tag="et", name="et")
                    nc.scalar.activation(et[:], stp[:], AF.Exp)
                    if k >= 4 * c:
                        nc.vector.tensor_mul(et[:], et[:],
                                             mask_sb[:, ts(k - 4 * c, SUB)])
                    if k == 0:
                        nc.vector.tensor_copy(acc[:], et[:])
                    else:
                        nc.vector.tensor_add(acc[:], acc[:], et[:])
                    ko, kj = divmod(k, 4)
                    ubase = ko * KCH + kj * 512
                    for cs in range(CT):
                        nc.tensor.matmul(
                            ht[cs][:],
                            xT_sb[:, ubase + cs * P: ubase + cs * P + P],
                            et[:], start=(k == 0), stop=(k == ntr - 1))
                if nq is not None and c == 0:
                    # chunk 1's projection can't dribble into chunk 0's
                    # short key loop -- its K chunk is still in flight then
                    for o in range(CT):
                        qproj_tile(1, nq, o)
                nc.tensor.matmul(sm[:], ones_sb[:], acc[:],
                                 start=True, stop=True)
                if nq is not None:
                    chunk_q[c + 1] = nq
                while fin:
                    fin.pop(0)()
                return ht, sm

            def finish_thunks(c, ht, sm, last=False):
                hs = hp.tile([P, CT, SUB], bf16, tag="hs", name="hs")
                r_sb = op.tile([P, SUB], f32r, tag="rsb", name="r_sb")
                og = ob.tile([P, CT, SUB], f32, tag="og", name="og")

                def copy_h():
                    # mid-pipeline ACT is busy with the next chunk's exps,
                    # so the copies run on DVE; for the final chunk both
                    # engines are idle and splitting halves the latency
                    for cs in range(CT):
                        if last and cs >= 2:
                            nc.scalar.activation(hs[:, cs, :], ht[cs][:],
                                                 AF.Identity)
                        else:
                            nc.vector.tensor_copy(hs[:, cs, :], ht[cs][:])

                def recip():
                    # row sums were accumulated broadcast across all
                    # partitions, so the reciprocal runs 128-way parallel and
                    # the result multiplies the output projection directly.
                    # Deferred off the chunk boundary so the copy_h burst is
                    # not delayed behind it on DVE.
                    with nc.allow_low_precision(reason="f32r is fp32-width"):
                        nc.vector.reciprocal(r_sb[:], sm[:])

                def proj_o(o):
                    pu = pp.tile([P, SUB], f32, tag="mm", name="pu")
                    for cj in range(CT):
                        nc.tensor.matmul(pu[:], wp_sb[:, cj, ts(o, P)],
                                         hs[:, cj, :],
                                         start=(cj == 0), stop=(cj == CT - 1))
                    nc.vector.tensor_mul(og[:, o, :], pu[:], r_sb[:])
                    if last:
                        # both DMA rings are idle at the tail: store each
                        # o-slice as soon as its normalize lands, alternating
                        # rings, so the drain overlaps the epilogue chain
                        eng = nc.sync if o % 2 else nc.scalar
                        eng.dma_start(out[:][:, o, ts(c, SUB)], og[:, o, :])
                    elif o == CT - 1:
                        # one batched store per chunk (4 KB per-partition
                        # rows, single dispatch on the otherwise-idle ring)
                        nc.scalar.dma_start(out[:][:, :, ts(c, SUB)], og[:])

                return [copy_h, recip] + \
                    [lambda o=o: proj_o(o) for o in range(CT)]

            warmup()
            qproj(0)
            fin = []
            for c in range(NCH):
                ht, sm = s_loop(c, fin)
                fin = finish_thunks(c, ht, sm, last=(c == NCH - 1))
            while fin:
                fin.pop(0)()

    nc.finalize()
    return nc


def _masks(h):
    m = np.zeros((4, P, SUB), np.float32)
    f = np.arange(SUB)[None, :]
    p = np.arange(P)[:, None]
    m[0] = (f >= p).astype(np.float32)
    m[1] = (f >= p + 128).astype(np.float32)
    if h == 1:
        m[2] = 1.0
        m[3] = 1.0
    return m


def _pmajor(w):
    # [C_out, C_in] weight (transposed use) -> [P, CT, C] partition-major
    return np.ascontiguousarray(
        w.T.reshape(CT, P, C).transpose(1, 0, 2))


def _in_maps(inputs):
    x = np.asarray(inputs["x"], np.float32)
    Wq = np.asarray(inputs["Wq"], np.float64)
    bq = np.asarray(inputs["bq"], np.float64)
    Wk = np.asarray(inputs["Wk"], np.float64)
    Wv = np.asarray(inputs["Wv"], np.float64)
    bv = np.asarray(inputs["bv"], np.float64)
    Wp = np.asarray(inputs["Wp"], np.float64)
    bp = np.asarray(inputs["bp"], np.float64)

    Wt = (Wk.T @ Wq) * SCALE           # folded Q~ weights (scale included)
    bt = (Wk.T @ bq) * SCALE           # folded Q~ bias
    W2 = Wp @ Wv                       # folded output projection
    b2 = (bp + Wp @ bv).astype(np.float32)   # host-side constant bias

    common = {
        "wqd": _pmajor(Wt.astype(np.float32)).astype(ml_dtypes.bfloat16),
        "wpd": _pmajor(W2.astype(np.float32)).astype(ml_dtypes.bfloat16),
        "scd": np.ascontiguousarray(
            bt.astype(np.float32).reshape(CT, P).T),
        "oned": np.ones((P, P), np.float32),
    }
    maps = []
    for core in range(NCORE):
        b, h = divmod(core, 2)
        # per-512-block permutation: this core's query half first
        perm = (np.arange(NCH)[:, None] * 512
                + (h * SUB + np.arange(512)[None, :]) % 512).ravel()
        cols = (np.arange(NCH)[:, None] * 512 + h * SUB
                + np.arange(SUB)[None, :]).ravel()
        xp = x[b][:, perm].astype(ml_dtypes.bfloat16)     # [C, T]
        m = dict(common)
        # [chan, t] layout, chunk-outer: xkd[sc][p, cj*512 + t'] =
        #   xp[cj*128+p, sc*512+t']
        m["xkd"] = np.ascontiguousarray(
            xp.reshape(CT, P, NCH, 512).transpose(2, 1, 0, 3))
        # [t, chan] layout, chunk-outer: xtd[sc][p, j*C + c] =
        #   xp[c, sc*512 + j*128 + p]
        m["xtd"] = np.ascontiguousarray(
            xp.T.reshape(NCH, 4, P, C).transpose(0, 2, 1, 3)
            .reshape(NCH, P, KCH))
        m["mkd"] = np.ascontiguousarray(
            _masks(h).transpose(1, 0, 2).reshape(P, 4 * SUB)
            .astype(ml_dtypes.bfloat16))
        maps.append((m, b, cols))
    return maps, b2


_prog_cache = {}


def _get_program():
    if "nc" not in _prog_cache:
        _prog_cache["nc"] = _build_program()
    return _prog_cache["nc"]


def kernel(**inputs):
    return _run(inputs, trace=False)[0]


def _run(inputs, trace=False):
    nc = _get_program()
    maps, b2 = _in_maps(inputs)
    res = run_bass_kernel_spmd(nc, [m for m, _, _ in maps],
                               core_ids=list(range(NCORE)), trace=trace)
    x = np.asarray(inputs["x"], np.float32)
    full = np.empty((B, C, T), np.float32)
    for core, (_, b, cols) in enumerate(maps):
        full[b][:, cols] = (res.results[core]["out"]
                            .transpose(1, 0, 2).reshape(C, TQ))
    # residual + folded constant bias, both in full f32 on the host
    full += x + b2[None, :, None]
    return full, res
